# revision 9
# baseline (speedup 1.0000x reference)
"""Trainium2 Bass kernel: 2-layer GCN (GCNConv -> ReLU -> GCNConv).

Math:  S = D^-1/2 (A + I) D^-1/2  (A from edge_index, self-loops appended)
       out = S @ relu(S @ x @ W1 + b1) @ W2 + b2
Using linearity, aggregate-then-matmul per layer with u = Dis*x:
       agg1 = A' @ u + u            (A' = adjacency without self-loops)
       h~   = Dis^2 * relu(agg1 @ W1 (+ b1))   (= Dis * h1, stored fp16)
       agg2 = A' @ h~ + h~
       out  = Dis * (agg2 @ W2) (+ b2)
The Dis row-scaling commutes with the dense matmul, so it is folded into a
single ScalarE activation per tile (scale = dis^2 resp. dis, func=relu/copy).

Distribution: nodes sharded over 8 NeuronCores.  Per layer, each core
scatter-adds incoming-edge source rows per 128-target tile with TensorE
matmuls against one-hot matrices (built on VectorE via is_equal vs iota, one
tile ahead of use), transposes via TensorE and applies the dense weight
matmul in fp16, all on a 2-deep software pipeline (PE order: aggregate(t),
transpose(t-1), dense(t-2)) so the TensorE stream never stalls.

Precision: the gathered/streamed neighbor rows, the one-hot matrices and the
AllGather payload are fp8e4 (self term, weights and dense path stay fp16;
measured HW rel-err 1.8e-2 vs the 2e-2 gate).  The scatter-add runs two fp8
chunks per PE pass with MatmulPerfMode.DoubleRow (2x fp8 throughput), so
both HBM traffic and aggregation PE time halve vs fp16.
  Layer 1: gather indices are static and the source data (fp8 Dis*x) is a
  kernel input, so the HOST pre-expands the gathered stream into edge order;
  the device streams it sequentially over HWDGE at full HBM bandwidth.
  Layer 2: fp8 activations are AllGathered into TWO window tensors (each
  int16-indexable: 8*window_rows <= 32767) so each tile needs only two
  dma_gather calls, but the collective itself is cut into ~8 small SLICES
  (separate input shard tensors, sliced output APs) fired as soon as each
  slice's tiles are computed — the CC stream pipelines behind layer-1
  compute and only a tiny final slice is exposed at the layer boundary.
  Gather descriptor generation cycles the 4 SWDGE queues in strict issue
  order (the tile scheduler's DMASW sem lanes are queue-locked); gather
  indices are pre-sorted for HBM locality.
"""

import os
import numpy as np
import ml_dtypes

NC_CORES = 8
TILE_P = 128
N_QUEUES = 4
WIN_TILES = 31  # 8 * 31 * 128 = 31744 <= 32767 (int16 gather index limit)
FP8 = ml_dtypes.float8_e4m3


def _round_up(v, m):
    return (v + m - 1) // m * m


def _windows(ntiles):
    """Gather windows (int16-indexable) as tile bounds.  Each window is
    also one AllGather (a Shared tensor has a single writer), so sizes
    taper: big early windows fire mid-layer-1 and their collectives hide
    behind compute; the late windows are small (quick collectives); the
    final 1-tile window keeps the exposed boundary latency tiny."""
    if ntiles <= 2:
        return [0, ntiles]
    b = [0]
    main = ntiles - 1  # reserve the 1-tile final window
    while main - b[-1] > 25:
        b.append(min(b[-1] + 25, main))
    rem = main - b[-1]
    if rem > 10:
        b.append(b[-1] + (rem * 2) // 3)
    b.append(main)
    b.append(ntiles)
    return sorted(set(b))


def _prep_host(x, edge_index):
    """Partition + pad the graph; build per-core stream/gather metadata."""
    x = np.asarray(x, dtype=np.float32)
    edge_index = np.asarray(edge_index)
    N, F = x.shape
    assert N % NC_CORES == 0, (N, NC_CORES)
    npc = N // NC_CORES
    npc_pad = _round_up(npc, TILE_P)
    ntiles = npc_pad // TILE_P
    n_pad = NC_CORES * npc_pad

    loops = np.arange(N, dtype=np.int64)
    # edges WITHOUT self-loops (self term handled on-device)
    row = edge_index[0].astype(np.int64)
    col = edge_index[1].astype(np.int64)
    # degree WITH self-loops (as the reference computes it)
    deg = np.bincount(np.concatenate([col, loops]), minlength=N).astype(np.float64)
    dis = np.where(deg > 0, 1.0 / np.sqrt(deg), 0.0).astype(np.float32)

    src_pad = (row // npc) * npc_pad + (row % npc)
    tgt_core = (col // npc).astype(np.int64)
    tgt_loc = col % npc
    tile_of = tgt_loc // TILE_P
    toff_of = (tgt_loc % TILE_P).astype(np.float32)

    # ---- layer 1: host-expanded stream, grouped by (core, tile) ----
    key1 = tgt_core * ntiles + tile_of
    cnt1 = np.bincount(key1, minlength=NC_CORES * ntiles).reshape(
        NC_CORES, ntiles)
    C1 = _round_up(cnt1.max(axis=0), TILE_P)  # padded counts [ntiles]
    tot1 = int(C1.sum())
    totch1 = tot1 // TILE_P
    nch1 = (C1 // TILE_P).astype(np.int64)
    cs1 = np.zeros(ntiles, dtype=np.int64)
    np.cumsum(nch1[:-1], out=cs1[1:])

    o1 = np.argsort(key1, kind="stable")
    src1_s, toff1_s = src_pad[o1], toff_of[o1]
    g1start = np.zeros(NC_CORES * ntiles + 1, dtype=np.int64)
    np.cumsum(cnt1.reshape(-1), out=g1start[1:])

    # ---- layer 2: dma_gather over W windows (= AllGather pieces) ----
    wb = _windows(ntiles)          # window bounds (tiles)
    W = len(wb) - 1
    asl = wb                       # AG slices coincide with windows
    NS = len(asl) - 1
    win_rows = [(wb[w + 1] - wb[w]) * TILE_P for w in range(W)]
    for r in win_rows:
        assert NC_CORES * r <= 32767, (win_rows,)
    slice_win = np.searchsorted(np.asarray(wb[1:W]),
                                np.asarray(asl[:-1]), side="right")

    src_core = row // npc
    src_loc = row % npc
    src_tile = src_loc // TILE_P
    win = np.searchsorted(np.asarray(wb[1:W]), src_tile, side="right")
    sl = np.searchsorted(np.asarray(asl[1:NS]), src_tile, side="right")
    asl_arr = np.asarray(asl)
    wb_arr = np.asarray(wb)
    rows_s = (asl_arr[sl + 1] - asl_arr[sl]) * TILE_P
    # global row inside the window tensor: slice block offset + core-major
    # position inside the slice
    gidx = (8 * (asl_arr[sl] - wb_arr[win]) * TILE_P
            + src_core * rows_s + (src_loc - asl_arr[sl] * TILE_P))

    key2 = (tgt_core * ntiles + tile_of) * W + win
    cnt2 = np.bincount(key2, minlength=NC_CORES * ntiles * W).reshape(
        NC_CORES, ntiles, W)
    C2 = cnt2.max(axis=0)
    C2 = np.where(C2 > 0, _round_up(C2, TILE_P), 0)  # [ntiles, W]
    tot2 = int(C2.sum())
    cW = (C2 // TILE_P).astype(np.int64)  # chunks per (tile, window)
    nch2 = cW.sum(axis=1)
    cs2 = np.zeros(ntiles, dtype=np.int64)
    np.cumsum(nch2[:-1], out=cs2[1:])

    # secondary sort by source index: the one-hot P absorbs any within-group
    # permutation, and ascending gather addresses improve HBM locality
    o2 = np.lexsort((gidx, key2))
    src2_s, toff2_s = gidx[o2], toff_of[o2]
    g2start = np.zeros(NC_CORES * ntiles * W + 1, dtype=np.int64)
    np.cumsum(cnt2.reshape(-1), out=g2start[1:])

    # padded Dis*x: fp8 copy feeds the edge streams, fp16 the self term
    xs32 = dis[:, None] * x
    xs_pad8 = np.zeros((NC_CORES, npc_pad, F), dtype=FP8)
    xs_pad8[:, :npc] = xs32.reshape(NC_CORES, npc, F).astype(FP8)
    xs_pad16 = np.zeros((NC_CORES, npc_pad, F), dtype=np.float16)
    xs_pad16[:, :npc] = xs32.reshape(NC_CORES, npc, F).astype(np.float16)
    xsl = np.ascontiguousarray(
        xs_pad16.reshape(NC_CORES, ntiles, TILE_P, F).transpose(0, 2, 1, 3))
    xs_flat8 = xs_pad8.reshape(n_pad, F)

    g1 = np.zeros((NC_CORES, 128, totch1, F), dtype=FP8)
    toff1 = np.full((NC_CORES, tot1), -1.0, dtype=np.float32)
    idx2 = np.zeros((NC_CORES, max(tot2, 16)), dtype=np.int16)
    toff2 = np.full((NC_CORES, max(tot2, TILE_P)), -1.0, dtype=np.float32)
    for p in range(NC_CORES):
        off = 0
        for t in range(ntiles):
            g = p * ntiles + t
            a, b = g1start[g], g1start[g + 1]
            n = b - a
            blk = g1[p, :, cs1[t]:cs1[t] + nch1[t], :]
            j = np.arange(n)
            # stream row j -> partition j%128, chunk j//128
            blk[j % 128, j // 128] = xs_flat8[src1_s[a:b]]
            toff1[p, off:off + n] = toff1_s[a:b]
            off += C1[t]
        off = 0
        for t in range(ntiles):
            for h in range(W):
                c = int(C2[t, h])
                if c == 0:
                    continue
                g = (p * ntiles + t) * W + h
                a, b = g2start[g], g2start[g + 1]
                n = b - a
                s = src2_s[a:b]
                assert n <= c and (s >= 0).all() and (s < 32767).all()
                idx2[p, off:off + n] = s.astype(np.int16)
                toff2[p, off:off + n] = toff2_s[a:b]
                off += c

    tot2c = max(tot2, 16)
    idx2_w = np.ascontiguousarray(
        np.tile(idx2.reshape(NC_CORES, tot2c // 16, 16).transpose(0, 2, 1),
                (1, 8, 1)))
    toff1_w = np.ascontiguousarray(
        toff1.reshape(NC_CORES, totch1, TILE_P).transpose(0, 2, 1)).astype(
            np.float16)
    tot2t = max(tot2, TILE_P)
    toff2_w = np.ascontiguousarray(
        toff2.reshape(NC_CORES, tot2t // TILE_P, TILE_P).transpose(0, 2, 1)
    ).astype(np.float16)

    dis_pad = np.zeros((NC_CORES, npc_pad), dtype=np.float32)
    dis_pad[:, :npc] = dis.reshape(NC_CORES, npc)
    dis_tiles = np.ascontiguousarray(
        dis_pad.reshape(NC_CORES, ntiles, TILE_P).transpose(0, 2, 1))
    dis2_tiles = np.ascontiguousarray(dis_tiles * dis_tiles)

    return dict(
        N=N, F=F, npc=npc, npc_pad=npc_pad, ntiles=ntiles, n_pad=n_pad,
        wb=wb, W=W, asl=asl, NS=NS, win_rows=win_rows, slice_win=slice_win,
        nch1=nch1, cs1=cs1, totch1=totch1,
        cW=cW, nch2=nch2, cs2=cs2, tot2=tot2,
        g1=g1.reshape(NC_CORES, 128, totch1 * F),
        xsl=xsl.reshape(NC_CORES, 128, ntiles * F),
        idx2=idx2_w, toff1=toff1_w, toff2=toff2_w,
        dis_tiles=dis_tiles, dis2_tiles=dis2_tiles,
    )


def _build_program(meta, has_b1, has_b2):
    import concourse.bacc as bacc
    import concourse.tile as tile
    from concourse import mybir

    F = meta["F"]
    ntiles = meta["ntiles"]
    npc_pad = meta["npc_pad"]
    wb, W, asl, NS = meta["wb"], meta["W"], meta["asl"], meta["NS"]
    win_rows, slice_win = meta["win_rows"], meta["slice_win"]
    nch1, cs1, totch1 = meta["nch1"], meta["cs1"], meta["totch1"]
    cW, nch2, cs2 = meta["cW"], meta["nch2"], meta["cs2"]
    totw2 = max(meta["tot2"], 16) // 16
    totch2 = max(meta["tot2"], TILE_P) // TILE_P
    nf = F // TILE_P
    f32, f16, i16 = mybir.dt.float32, mybir.dt.float16, mybir.dt.int16
    f8 = mybir.dt.float8e4
    AF = mybir.ActivationFunctionType
    DR = mybir.MatmulPerfMode.DoubleRow

    nc = bacc.Bacc("TRN2", target_bir_lowering=False, debug=False,
                   num_devices=NC_CORES, num_swdge_queues=N_QUEUES)

    g1_d = nc.dram_tensor("g1", [128, totch1 * F], f8, kind="ExternalInput")
    xsl_d = nc.dram_tensor("xsl", [128, ntiles * F], f16, kind="ExternalInput")
    idx_d = nc.dram_tensor("idx", [128, totw2], i16, kind="ExternalInput")
    toff1_d = nc.dram_tensor("toff1", [128, totch1], f16, kind="ExternalInput")
    toff2_d = nc.dram_tensor("toff2", [128, totch2], f16, kind="ExternalInput")
    dis_d = nc.dram_tensor("dis", [128, ntiles], f32, kind="ExternalInput")
    dis2_d = nc.dram_tensor("dis2", [128, ntiles], f32, kind="ExternalInput")
    w1_d = nc.dram_tensor("w1", [F, F], f16, kind="ExternalInput")
    w2_d = nc.dram_tensor("w2", [F, F], f16, kind="ExternalInput")
    id16_d = nc.dram_tensor("id16", [128, 128], f16, kind="ExternalInput")
    iota_d = nc.dram_tensor("iota", [128, 128], f16, kind="ExternalInput")
    if has_b1:
        b1_d = nc.dram_tensor("b1r", [128, F], f32, kind="ExternalInput")
    if has_b2:
        b2_d = nc.dram_tensor("b2r", [128, F], f32, kind="ExternalInput")
    out_d = nc.dram_tensor("out", [npc_pad, F], f16, kind="ExternalOutput")

    eq, add = mybir.AluOpType.is_equal, mybir.AluOpType.add

    with tile.TileContext(nc) as tc:
        with (
            tc.tile_pool(name="const", bufs=1) as cpool,
            tc.tile_pool(name="gbuf", bufs=4) as gpool,
            tc.tile_pool(name="gpre", bufs=3) as prepool,
            tc.tile_pool(name="pbuf", bufs=3) as ppool,
            tc.tile_pool(name="work", bufs=4) as wpool,
            tc.tile_pool(name="h8buf", bufs=3) as hpool,
            tc.tile_pool(name="psA", bufs=3, space="PSUM") as psa,
            tc.tile_pool(name="psB", bufs=2, space="PSUM") as psb,
            tc.tile_pool(name="psC", bufs=3, space="PSUM") as psc,
            tc.tile_pool(name="dram", bufs=1, space="DRAM") as dpool,
        ):
            idx_sb = cpool.tile([128, totw2], i16)
            nc.sync.dma_start(idx_sb[:], idx_d[:, :])
            toff1_sb = cpool.tile([128, totch1], f16)
            nc.sync.dma_start(toff1_sb[:], toff1_d[:, :])
            toff2_sb = cpool.tile([128, totch2], f16)
            nc.sync.dma_start(toff2_sb[:], toff2_d[:, :])
            dis_sb = cpool.tile([128, ntiles], f32)
            nc.sync.dma_start(dis_sb[:], dis_d[:, :])
            dis2_sb = cpool.tile([128, ntiles], f32)
            nc.sync.dma_start(dis2_sb[:], dis2_d[:, :])
            id16_sb = cpool.tile([128, 128], f16)
            nc.sync.dma_start(id16_sb[:], id16_d[:, :])
            iota_sb = cpool.tile([128, 128], f16)
            nc.sync.dma_start(iota_sb[:], iota_d[:, :])
            w1_sb = cpool.tile([128, nf, F], f16)
            w2_sb = cpool.tile([128, nf, F], f16)
            for i in range(nf):
                nc.sync.dma_start(w1_sb[:, i, :], w1_d[128 * i:128 * (i + 1), :])
                nc.sync.dma_start(w2_sb[:, i, :], w2_d[128 * i:128 * (i + 1), :])
            if has_b1:
                b1_sb = cpool.tile([128, F], f32)
                nc.sync.dma_start(b1_sb[:], b1_d[:, :])
            if has_b2:
                b2_sb = cpool.tile([128, F], f32)
                nc.sync.dma_start(b2_sb[:], b2_d[:, :])

            # local shard, fp16: holds Dis*x during layer 1, then Dis*h1
            self_sb = cpool.tile([128, ntiles, F], f16)
            nc.sync.dma_start(
                self_sb[:], xsl_d[:, :].rearrange("p (t f) -> p t f", f=F))

            # per-AG-slice input shards (separate tensors: a slice is only
            # written by its own tiles, so firing its AllGather never
            # serializes against later hs writes), per-WINDOW gather sources
            sl_rows = [(asl[s + 1] - asl[s]) * TILE_P for s in range(NS)]
            hs_shard = [dpool.tile([sl_rows[s], F], f8, name=f"hs_shard{s}")
                        for s in range(NS)]
            hs_win = [dpool.tile([NC_CORES * win_rows[w], F], f8,
                                 addr_space="Shared", name=f"hs_win{w}")
                      for w in range(W)]
            # AG slice -> tile index whose stage_b fires it
            ag_fire = {asl[s + 1] - 1: s for s in range(NS)}
            # tile -> AG slice
            tile_slice = np.searchsorted(
                np.asarray(asl[1:NS]), np.arange(ntiles), side="right")

            def fire_ag(s):
                w = int(slice_win[s])
                off = 8 * (asl[s] - wb[w]) * TILE_P
                nc.gpsimd.collective_compute(
                    "AllGather", mybir.AluOpType.bypass,
                    replica_groups=[list(range(NC_CORES))],
                    ins=[hs_shard[s].opt()],
                    outs=[hs_win[w][off:off + NC_CORES * sl_rows[s], :].opt()])

            PRE = 3  # tiles whose gathers are issued before the main loop
            g_pend = {}
            g_done = {}
            # SWDGE queue must advance in lockstep with issue order: the tile
            # scheduler hands out DMASW sem lanes round-robin per SWDGE
            # instruction, and each sem is locked to one queue — a strict
            # global cycle keeps lane<->queue consistent.
            gq = [0]

            def gather_win(t, w, G):
                cnt = int(cW[t, w])
                if cnt == 0:
                    return
                o_rel = int(cW[t, :w].sum())
                cs = int(cs2[t]) + o_rel
                q = gq[0] % N_QUEUES
                gq[0] += 1
                nc.gpsimd.dma_gather(
                    G[:, o_rel:o_rel + cnt, :], hs_win[w][:, :],
                    idx_sb[:, cs * 8:(cs + cnt) * 8],
                    cnt * 128, cnt * 128, F,
                    single_packet=(cnt * 128 <= 128),
                    queue_num=q)

            def agg_matmuls(aggp, P, G, t, nch):
                """scatter-add: self term (fp16) + fp8 DoubleRow chunk pairs."""
                nc.tensor.matmul(aggp[:], id16_sb[:], self_sb[:, t, :],
                                 start=True, stop=(nch == 0))
                c = 0
                while c < nch:
                    if c + 2 <= nch:
                        nc.tensor.matmul(aggp[:], P[:, c:c + 2, :],
                                         G[:, c:c + 2, :], start=False,
                                         stop=(c + 2 == nch), perf_mode=DR)
                        c += 2
                    else:
                        nc.tensor.matmul(aggp[:], P[:, c, :], G[:, c, :],
                                         start=False, stop=True)
                        c += 1

            for layer in range(2):
                w_sb = w1_sb if layer == 0 else w2_sb
                toff_sb = toff1_sb if layer == 0 else toff2_sb

                def build_p(t):
                    """one-hot matrices for tile t (VectorE), built one tile
                    ahead so the PE never waits on them."""
                    if layer == 0:
                        nch, cs = int(nch1[t]), int(cs1[t])
                    else:
                        nch, cs = int(nch2[t]), int(cs2[t])
                    if not nch:
                        return None
                    P = ppool.tile([128, nch, 128], f8, tag="P")
                    nc.vector.tensor_tensor(
                        P[:],
                        iota_sb[:].unsqueeze(1).broadcast_to([128, nch, 128]),
                        toff_sb[:, cs:cs + nch].unsqueeze(2).broadcast_to(
                            [128, nch, 128]),
                        eq)
                    return P

                def stage_a(t, P):
                    """gather/stream G, scatter-add the incoming messages +
                    self term into PSUM, copy to SBUF (ScalarE)."""
                    if layer == 0:
                        nch, cs = int(nch1[t]), int(cs1[t])
                        G = gpool.tile([128, max(nch, 1), F], f8, tag="G")
                        if nch:
                            nc.sync.dma_start(
                                G[:, 0:nch, :],
                                g1_d[:, cs * F:(cs + nch) * F].rearrange(
                                    "p (c f) -> p c f", f=F))
                    else:
                        nch = int(nch2[t])
                        if t in g_pend:
                            G = g_pend.pop(t)
                            done = g_done.pop(t)
                        else:
                            G = gpool.tile([128, max(nch, 1), F], f8,
                                           tag="G")
                            done = ()
                        for w in range(W):
                            if w not in done:
                                gather_win(t, w, G)
                    # scatter-add (+ self term via identity weights)
                    aggp = psa.tile([128, F], f32, tag="aggp")
                    agg_matmuls(aggp, P, G, t, nch)
                    # PSUM -> SBUF f16 (ScalarE; Dis scaling folded into the
                    # final activation instead)
                    aggc = wpool.tile([128, F], f16, tag="aggc")
                    nc.scalar.activation(aggc[:], aggp[:], AF.Copy)
                    return aggc

                def stage_t(t, aggc):
                    """TensorE transpose of the aggregate + copy out of PSUM."""
                    pT = psb.tile([128, F], f16, tag="pT")
                    for i in range(nf):
                        nc.tensor.transpose(pT[:, 128 * i:128 * (i + 1)],
                                            aggc[:, 128 * i:128 * (i + 1)],
                                            id16_sb[:])
                    aggT = wpool.tile([128, nf, 128], f16, tag="aggT")
                    nc.vector.tensor_copy(
                        aggT[:].rearrange("p a b -> p (a b)"), pT[:])
                    return aggT

                def stage_b(t, aggT):
                    """dense weight matmul + scaled activation + writeback."""
                    zp = psc.tile([128, F], f32, tag="zp")
                    for i in range(nf):
                        nc.tensor.matmul(zp[:], aggT[:, i, :], w_sb[:, i, :],
                                         start=(i == 0), stop=(i == nf - 1))
                    r0, r1 = TILE_P * t, TILE_P * (t + 1)
                    if layer == 0:
                        zin = zp[:]
                        if has_b1:
                            zb = wpool.tile([128, F], f32, tag="zb")
                            nc.vector.tensor_tensor(zb[:], zp[:], b1_sb[:], add)
                            zin = zb[:]
                        # self_sb[t] := dis^2 * relu(z) == dis * relu(dis * z)
                        nc.scalar.activation(self_sb[:, t, :], zin, AF.Relu,
                                             scale=dis2_sb[:, t:t + 1])
                        # fp8 copy of the same activation for the AllGather /
                        # layer-2 gather stream
                        h8 = hpool.tile([128, F], f8, tag="h8")
                        nc.scalar.activation(h8[:], zin, AF.Relu,
                                             scale=dis2_sb[:, t:t + 1])
                        s = int(tile_slice[t])
                        b0 = (t - asl[s]) * TILE_P
                        # hs writes ride the ScalarE HWDGE queue: off the
                        # SWDGE lanes (whose sem rotation the gathers own)
                        # and off the SP ring (so the g1 stream never waits)
                        nc.scalar.dma_start(hs_shard[s][b0:b0 + TILE_P, :],
                                            h8[:])
                        if t in ag_fire:
                            k = ag_fire[t]
                            fire_ag(k)
                            # prefetch the first tiles' gathers once their
                            # window's AllGather slices have all fired: the
                            # non-final windows at the second-to-last fire
                            # (their data landed long ago — no engine-
                            # blocking waits before the final AG fires), the
                            # last window right after the final fire.  A
                            # dedicated pool keeps these allocations out of
                            # the main G ring.
                            if k == NS - 2 and NS >= 2 and \
                                    asl[k + 1] >= ntiles - 1:
                                for tt in range(min(PRE, ntiles)):
                                    nch_t = int(nch2[tt])
                                    G = prepool.tile([128, max(nch_t, 1), F],
                                                     f8, tag="Gpre")
                                    g_pend[tt] = G
                                    g_done[tt] = set()
                                    # only windows whose AllGather landed
                                    # long ago: a not-yet-landed window
                                    # would block the gpsimd engine before
                                    # the final AllGather fires
                                    for w in range(max(W - 3, 0)):
                                        gather_win(tt, w, G)
                                        g_done[tt].add(w)
                            elif k == NS - 1:
                                if not g_pend:
                                    for tt in range(min(PRE, ntiles)):
                                        nch_t = int(nch2[tt])
                                        G = prepool.tile(
                                            [128, max(nch_t, 1), F],
                                            f8, tag="Gpre")
                                        g_pend[tt] = G
                                        g_done[tt] = set()
                                for tt in sorted(g_pend):
                                    for w in range(W):
                                        if w not in g_done[tt]:
                                            gather_win(tt, w, g_pend[tt])
                                            g_done[tt].add(w)
                    else:
                        o_t = wpool.tile([128, F], f16, tag="ot")
                        zin = zp[:]
                        if has_b2:
                            zb = wpool.tile([128, F], f32, tag="zb")
                            nc.vector.tensor_tensor(zb[:], zp[:], b2_sb[:], add)
                            zin = zb[:]
                        # out := dis * z  (SP ring is idle in layer 2)
                        nc.scalar.activation(o_t[:], zin, AF.Copy,
                                             scale=dis_sb[:, t:t + 1])
                        nc.sync.dma_start(out_d[r0:r1, :], o_t[:])

                # 2-deep software pipeline: PE order is aggp(t), T(t-1),
                # zp(t-2) so the TensorE stream never stalls on the
                # cross-engine transpose round-trip; P built one tile ahead
                p_next = build_p(0)
                aggc_q, aggt_q = {}, {}
                for t in range(ntiles + 2):
                    if t < ntiles:
                        P_cur = p_next
                        p_next = build_p(t + 1) if t + 1 < ntiles else None
                        aggc_q[t] = stage_a(t, P_cur)
                    if 1 <= t <= ntiles:
                        aggt_q[t - 1] = stage_t(t - 1, aggc_q.pop(t - 1))
                    if t >= 2:
                        stage_b(t - 2, aggt_q.pop(t - 2))

    nc.compile()
    return nc


def kernel(x, edge_index, W1, b1, W2, b2):
    x = np.asarray(x, dtype=np.float32)
    W1 = np.asarray(W1, dtype=np.float32)
    W2 = np.asarray(W2, dtype=np.float32)
    b1 = np.asarray(b1, dtype=np.float32)
    b2 = np.asarray(b2, dtype=np.float32)
    meta = _prep_host(x, edge_index)

    has_b1 = bool(np.any(b1))
    has_b2 = bool(np.any(b2))
    nc = _build_program(meta, has_b1, has_b2)

    in_maps = []
    for p in range(NC_CORES):
        m = {
            "g1": meta["g1"][p],
            "xsl": meta["xsl"][p],
            "idx": meta["idx2"][p],
            "toff1": meta["toff1"][p],
            "toff2": meta["toff2"][p],
            "dis": meta["dis_tiles"][p],
            "dis2": meta["dis2_tiles"][p],
            "w1": W1.astype(np.float16), "w2": W2.astype(np.float16),
            "id16": np.eye(128, dtype=np.float16),
            "iota": np.tile(np.arange(128, dtype=np.float16), (128, 1)),
        }
        if has_b1:
            m["b1r"] = np.tile(b1, (128, 1)).astype(np.float32)
        if has_b2:
            m["b2r"] = np.tile(b2, (128, 1)).astype(np.float32)
        in_maps.append(m)

    if os.environ.get("GNN_SIM", "0") == "1":
        from concourse.bass_interp import MultiCoreSim
        sim = MultiCoreSim(nc, num_cores=NC_CORES, trace=False)
        cores = list(sim.cores.values())
        for p, core in enumerate(cores):
            for k, v in in_maps[p].items():
                core.tensor(k)[:] = v
        sim.simulate(check_with_hw=False)
        shards = [cores[p].tensor("out").copy() for p in range(NC_CORES)]
    else:
        from concourse import bass_utils
        trace = os.environ.get("GNN_TRACE", "0") == "1"
        res = bass_utils.run_bass_kernel_spmd(
            nc, in_maps, core_ids=list(range(NC_CORES)), trace=trace)
        if trace and res.exec_time_ns is not None:
            print(f"HW exec time: {res.exec_time_ns} ns")
        kernel.last_results = res
        shards = [res.results[p]["out"] for p in range(NC_CORES)]

    npc = meta["npc"]
    out = np.concatenate([s[:npc] for s in shards], axis=0)
    return out.astype(np.float32)


# revision 10
# speedup vs baseline: 1.0142x; 1.0142x over previous
"""Trainium2 Bass kernel: 2-layer GCN (GCNConv -> ReLU -> GCNConv).

Math:  S = D^-1/2 (A + I) D^-1/2  (A from edge_index, self-loops appended)
       out = S @ relu(S @ x @ W1 + b1) @ W2 + b2
Using linearity, aggregate-then-matmul per layer with u = Dis*x:
       agg1 = A' @ u + u            (A' = adjacency without self-loops)
       h~   = Dis^2 * relu(agg1 @ W1 (+ b1))   (= Dis * h1, stored fp16)
       agg2 = A' @ h~ + h~
       out  = Dis * (agg2 @ W2) (+ b2)
The Dis row-scaling commutes with the dense matmul, so it is folded into a
single ScalarE activation per tile (scale = dis^2 resp. dis, func=relu/copy).

Distribution: nodes sharded over 8 NeuronCores.  Per layer, each core
scatter-adds incoming-edge source rows per 128-target tile with TensorE
matmuls against one-hot matrices (built on VectorE via is_equal vs iota, one
tile ahead of use), transposes via TensorE and applies the dense weight
matmul in fp16, all on a 2-deep software pipeline (PE order: aggregate(t),
transpose(t-1), dense(t-2)) so the TensorE stream never stalls.

Precision: the gathered/streamed neighbor rows, the one-hot matrices and the
AllGather payload are fp8e4 (self term, weights and dense path stay fp16;
measured HW rel-err 1.8e-2 vs the 2e-2 gate).  The scatter-add runs two fp8
chunks per PE pass with MatmulPerfMode.DoubleRow (2x fp8 throughput), so
both HBM traffic and aggregation PE time halve vs fp16.
  Layer 1: gather indices are static and the source data (fp8 Dis*x) is a
  kernel input, so the HOST pre-expands the gathered stream into edge order;
  the device streams it sequentially over HWDGE at full HBM bandwidth.
  Layer 2: fp8 activations are AllGathered into TWO window tensors (each
  int16-indexable: 8*window_rows <= 32767) so each tile needs only two
  dma_gather calls, but the collective itself is cut into ~8 small SLICES
  (separate input shard tensors, sliced output APs) fired as soon as each
  slice's tiles are computed — the CC stream pipelines behind layer-1
  compute and only a tiny final slice is exposed at the layer boundary.
  Gather descriptor generation cycles the 4 SWDGE queues in strict issue
  order (the tile scheduler's DMASW sem lanes are queue-locked); gather
  indices are pre-sorted for HBM locality.
"""

import os
import numpy as np
import ml_dtypes

NC_CORES = 8
TILE_P = 128
N_QUEUES = 4
WIN_TILES = 31  # 8 * 31 * 128 = 31744 <= 32767 (int16 gather index limit)
FP8 = ml_dtypes.float8_e4m3


def _round_up(v, m):
    return (v + m - 1) // m * m


def _windows(ntiles):
    """Gather windows (int16-indexable) as tile bounds.  Each window is
    also one AllGather (a Shared tensor has a single writer), so sizes
    taper: big early windows fire mid-layer-1 and their collectives hide
    behind compute; the late windows are small (quick collectives); the
    final 1-tile window keeps the exposed boundary latency tiny."""
    if ntiles <= 2:
        return [0, ntiles]
    b = [0]
    main = ntiles - 1  # reserve the 1-tile final window
    while main - b[-1] > 25:
        b.append(min(b[-1] + 25, main))
    rem = main - b[-1]
    if rem > 10:
        b.append(b[-1] + (rem * 2) // 3)
    b.append(main)
    b.append(ntiles)
    return sorted(set(b))


def _prep_host(x, edge_index):
    """Partition + pad the graph; build per-core stream/gather metadata."""
    x = np.asarray(x, dtype=np.float32)
    edge_index = np.asarray(edge_index)
    N, F = x.shape
    assert N % NC_CORES == 0, (N, NC_CORES)
    npc = N // NC_CORES
    npc_pad = _round_up(npc, TILE_P)
    ntiles = npc_pad // TILE_P
    n_pad = NC_CORES * npc_pad

    loops = np.arange(N, dtype=np.int64)
    # edges WITHOUT self-loops (self term handled on-device)
    row = edge_index[0].astype(np.int64)
    col = edge_index[1].astype(np.int64)
    # degree WITH self-loops (as the reference computes it)
    deg = np.bincount(np.concatenate([col, loops]), minlength=N).astype(np.float64)
    dis = np.where(deg > 0, 1.0 / np.sqrt(deg), 0.0).astype(np.float32)

    src_pad = (row // npc) * npc_pad + (row % npc)
    tgt_core = (col // npc).astype(np.int64)
    tgt_loc = col % npc
    tile_of = tgt_loc // TILE_P
    toff_of = (tgt_loc % TILE_P).astype(np.float32)

    # ---- layer 1: host-expanded stream, grouped by (core, tile) ----
    key1 = tgt_core * ntiles + tile_of
    cnt1 = np.bincount(key1, minlength=NC_CORES * ntiles).reshape(
        NC_CORES, ntiles)
    C1 = _round_up(cnt1.max(axis=0), TILE_P)  # padded counts [ntiles]
    tot1 = int(C1.sum())
    totch1 = tot1 // TILE_P
    nch1 = (C1 // TILE_P).astype(np.int64)
    cs1 = np.zeros(ntiles, dtype=np.int64)
    np.cumsum(nch1[:-1], out=cs1[1:])

    o1 = np.argsort(key1, kind="stable")
    src1_s, toff1_s = src_pad[o1], toff_of[o1]
    g1start = np.zeros(NC_CORES * ntiles + 1, dtype=np.int64)
    np.cumsum(cnt1.reshape(-1), out=g1start[1:])

    # ---- layer 2: dma_gather over W windows (= AllGather pieces) ----
    wb = _windows(ntiles)          # window bounds (tiles)
    W = len(wb) - 1
    asl = wb                       # AG slices coincide with windows
    NS = len(asl) - 1
    win_rows = [(wb[w + 1] - wb[w]) * TILE_P for w in range(W)]
    for r in win_rows:
        assert NC_CORES * r <= 32767, (win_rows,)
    slice_win = np.searchsorted(np.asarray(wb[1:W]),
                                np.asarray(asl[:-1]), side="right")

    src_core = row // npc
    src_loc = row % npc
    src_tile = src_loc // TILE_P
    win = np.searchsorted(np.asarray(wb[1:W]), src_tile, side="right")
    sl = np.searchsorted(np.asarray(asl[1:NS]), src_tile, side="right")
    asl_arr = np.asarray(asl)
    wb_arr = np.asarray(wb)
    rows_s = (asl_arr[sl + 1] - asl_arr[sl]) * TILE_P
    # global row inside the window tensor: slice block offset + core-major
    # position inside the slice
    gidx = (8 * (asl_arr[sl] - wb_arr[win]) * TILE_P
            + src_core * rows_s + (src_loc - asl_arr[sl] * TILE_P))

    key2 = (tgt_core * ntiles + tile_of) * W + win
    cnt2 = np.bincount(key2, minlength=NC_CORES * ntiles * W).reshape(
        NC_CORES, ntiles, W)
    C2 = cnt2.max(axis=0)
    C2 = np.where(C2 > 0, _round_up(C2, TILE_P), 0)  # [ntiles, W]
    tot2 = int(C2.sum())
    cW = (C2 // TILE_P).astype(np.int64)  # chunks per (tile, window)
    nch2 = cW.sum(axis=1)
    cs2 = np.zeros(ntiles, dtype=np.int64)
    np.cumsum(nch2[:-1], out=cs2[1:])

    # secondary sort by source index: the one-hot P absorbs any within-group
    # permutation, and ascending gather addresses improve HBM locality
    o2 = np.lexsort((gidx, key2))
    src2_s, toff2_s = gidx[o2], toff_of[o2]
    g2start = np.zeros(NC_CORES * ntiles * W + 1, dtype=np.int64)
    np.cumsum(cnt2.reshape(-1), out=g2start[1:])

    # padded Dis*x: fp8 copy feeds the edge streams, fp16 the self term
    xs32 = dis[:, None] * x
    xs_pad8 = np.zeros((NC_CORES, npc_pad, F), dtype=FP8)
    xs_pad8[:, :npc] = xs32.reshape(NC_CORES, npc, F).astype(FP8)
    xs_pad16 = np.zeros((NC_CORES, npc_pad, F), dtype=np.float16)
    xs_pad16[:, :npc] = xs32.reshape(NC_CORES, npc, F).astype(np.float16)
    xsl = np.ascontiguousarray(
        xs_pad16.reshape(NC_CORES, ntiles, TILE_P, F).transpose(0, 2, 1, 3))
    xs_flat8 = xs_pad8.reshape(n_pad, F)

    g1 = np.zeros((NC_CORES, 128, totch1, F), dtype=FP8)
    toff1 = np.full((NC_CORES, tot1), -1.0, dtype=np.float32)
    idx2 = np.zeros((NC_CORES, max(tot2, 16)), dtype=np.int16)
    toff2 = np.full((NC_CORES, max(tot2, TILE_P)), -1.0, dtype=np.float32)
    for p in range(NC_CORES):
        off = 0
        for t in range(ntiles):
            g = p * ntiles + t
            a, b = g1start[g], g1start[g + 1]
            n = b - a
            blk = g1[p, :, cs1[t]:cs1[t] + nch1[t], :]
            j = np.arange(n)
            # stream row j -> partition j%128, chunk j//128
            blk[j % 128, j // 128] = xs_flat8[src1_s[a:b]]
            toff1[p, off:off + n] = toff1_s[a:b]
            off += C1[t]
        off = 0
        for t in range(ntiles):
            for h in range(W):
                c = int(C2[t, h])
                if c == 0:
                    continue
                g = (p * ntiles + t) * W + h
                a, b = g2start[g], g2start[g + 1]
                n = b - a
                s = src2_s[a:b]
                assert n <= c and (s >= 0).all() and (s < 32767).all()
                idx2[p, off:off + n] = s.astype(np.int16)
                toff2[p, off:off + n] = toff2_s[a:b]
                off += c

    tot2c = max(tot2, 16)
    idx2_w = np.ascontiguousarray(
        np.tile(idx2.reshape(NC_CORES, tot2c // 16, 16).transpose(0, 2, 1),
                (1, 8, 1)))
    toff1_w = np.ascontiguousarray(
        toff1.reshape(NC_CORES, totch1, TILE_P).transpose(0, 2, 1)).astype(
            np.float16)
    tot2t = max(tot2, TILE_P)
    toff2_w = np.ascontiguousarray(
        toff2.reshape(NC_CORES, tot2t // TILE_P, TILE_P).transpose(0, 2, 1)
    ).astype(np.float16)

    dis_pad = np.zeros((NC_CORES, npc_pad), dtype=np.float32)
    dis_pad[:, :npc] = dis.reshape(NC_CORES, npc)
    dis_tiles = np.ascontiguousarray(
        dis_pad.reshape(NC_CORES, ntiles, TILE_P).transpose(0, 2, 1))
    dis2_tiles = np.ascontiguousarray(dis_tiles * dis_tiles)

    return dict(
        N=N, F=F, npc=npc, npc_pad=npc_pad, ntiles=ntiles, n_pad=n_pad,
        wb=wb, W=W, asl=asl, NS=NS, win_rows=win_rows, slice_win=slice_win,
        nch1=nch1, cs1=cs1, totch1=totch1,
        cW=cW, nch2=nch2, cs2=cs2, tot2=tot2,
        g1=g1.reshape(NC_CORES, 128, totch1 * F),
        xsl=xsl.reshape(NC_CORES, 128, ntiles * F),
        idx2=idx2_w, toff1=toff1_w, toff2=toff2_w,
        dis_tiles=dis_tiles, dis2_tiles=dis2_tiles,
    )


def _build_program(meta, has_b1, has_b2):
    import concourse.bacc as bacc
    import concourse.tile as tile
    from concourse import mybir

    F = meta["F"]
    ntiles = meta["ntiles"]
    npc_pad = meta["npc_pad"]
    wb, W, asl, NS = meta["wb"], meta["W"], meta["asl"], meta["NS"]
    win_rows, slice_win = meta["win_rows"], meta["slice_win"]
    nch1, cs1, totch1 = meta["nch1"], meta["cs1"], meta["totch1"]
    cW, nch2, cs2 = meta["cW"], meta["nch2"], meta["cs2"]
    totw2 = max(meta["tot2"], 16) // 16
    totch2 = max(meta["tot2"], TILE_P) // TILE_P
    nf = F // TILE_P
    f32, f16, i16 = mybir.dt.float32, mybir.dt.float16, mybir.dt.int16
    f8 = mybir.dt.float8e4
    AF = mybir.ActivationFunctionType
    DR = mybir.MatmulPerfMode.DoubleRow

    nc = bacc.Bacc("TRN2", target_bir_lowering=False, debug=False,
                   num_devices=NC_CORES, num_swdge_queues=N_QUEUES)

    g1_d = nc.dram_tensor("g1", [128, totch1 * F], f8, kind="ExternalInput")
    xsl_d = nc.dram_tensor("xsl", [128, ntiles * F], f16, kind="ExternalInput")
    idx_d = nc.dram_tensor("idx", [128, totw2], i16, kind="ExternalInput")
    toff1_d = nc.dram_tensor("toff1", [128, totch1], f16, kind="ExternalInput")
    toff2_d = nc.dram_tensor("toff2", [128, totch2], f16, kind="ExternalInput")
    dis_d = nc.dram_tensor("dis", [128, ntiles], f32, kind="ExternalInput")
    dis2_d = nc.dram_tensor("dis2", [128, ntiles], f32, kind="ExternalInput")
    w1_d = nc.dram_tensor("w1", [F, F], f16, kind="ExternalInput")
    w2_d = nc.dram_tensor("w2", [F, F], f16, kind="ExternalInput")
    id16_d = nc.dram_tensor("id16", [128, 128], f16, kind="ExternalInput")
    iota_d = nc.dram_tensor("iota", [128, 128], f16, kind="ExternalInput")
    if has_b1:
        b1_d = nc.dram_tensor("b1r", [128, F], f32, kind="ExternalInput")
    if has_b2:
        b2_d = nc.dram_tensor("b2r", [128, F], f32, kind="ExternalInput")
    out_d = nc.dram_tensor("out", [npc_pad, F], f16, kind="ExternalOutput")

    eq, add = mybir.AluOpType.is_equal, mybir.AluOpType.add

    with tile.TileContext(nc) as tc:
        with (
            tc.tile_pool(name="const", bufs=1) as cpool,
            tc.tile_pool(name="gbuf", bufs=4) as gpool,
            tc.tile_pool(name="gpre", bufs=3) as prepool,
            tc.tile_pool(name="pbuf", bufs=3) as ppool,
            tc.tile_pool(name="work", bufs=4) as wpool,
            tc.tile_pool(name="h8buf", bufs=3) as hpool,
            tc.tile_pool(name="psA", bufs=3, space="PSUM") as psa,
            tc.tile_pool(name="psB", bufs=2, space="PSUM") as psb,
            tc.tile_pool(name="psC", bufs=3, space="PSUM") as psc,
            tc.tile_pool(name="dram", bufs=1, space="DRAM") as dpool,
        ):
            idx_sb = cpool.tile([128, totw2], i16)
            nc.sync.dma_start(idx_sb[:], idx_d[:, :])
            toff1_sb = cpool.tile([128, totch1], f16)
            nc.sync.dma_start(toff1_sb[:], toff1_d[:, :])
            toff2_sb = cpool.tile([128, totch2], f16)
            nc.sync.dma_start(toff2_sb[:], toff2_d[:, :])
            dis_sb = cpool.tile([128, ntiles], f32)
            nc.sync.dma_start(dis_sb[:], dis_d[:, :])
            dis2_sb = cpool.tile([128, ntiles], f32)
            nc.sync.dma_start(dis2_sb[:], dis2_d[:, :])
            id16_sb = cpool.tile([128, 128], f16)
            nc.sync.dma_start(id16_sb[:], id16_d[:, :])
            iota_sb = cpool.tile([128, 128], f16)
            nc.sync.dma_start(iota_sb[:], iota_d[:, :])
            w1_sb = cpool.tile([128, nf, F], f16)
            w2_sb = cpool.tile([128, nf, F], f16)
            for i in range(nf):
                nc.sync.dma_start(w1_sb[:, i, :], w1_d[128 * i:128 * (i + 1), :])
                nc.sync.dma_start(w2_sb[:, i, :], w2_d[128 * i:128 * (i + 1), :])
            if has_b1:
                b1_sb = cpool.tile([128, F], f32)
                nc.sync.dma_start(b1_sb[:], b1_d[:, :])
            if has_b2:
                b2_sb = cpool.tile([128, F], f32)
                nc.sync.dma_start(b2_sb[:], b2_d[:, :])

            # local shard, fp16: holds Dis*x during layer 1, then Dis*h1
            self_sb = cpool.tile([128, ntiles, F], f16)
            nc.sync.dma_start(
                self_sb[:], xsl_d[:, :].rearrange("p (t f) -> p t f", f=F))

            # per-AG-slice input shards (separate tensors: a slice is only
            # written by its own tiles, so firing its AllGather never
            # serializes against later hs writes), per-WINDOW gather sources
            sl_rows = [(asl[s + 1] - asl[s]) * TILE_P for s in range(NS)]
            hs_shard = [dpool.tile([sl_rows[s], F], f8, name=f"hs_shard{s}")
                        for s in range(NS)]
            hs_win = [dpool.tile([NC_CORES * win_rows[w], F], f8,
                                 addr_space="Shared", name=f"hs_win{w}")
                      for w in range(W)]
            # AG slice -> tile index whose stage_b fires it
            ag_fire = {asl[s + 1] - 1: s for s in range(NS)}
            # tile -> AG slice
            tile_slice = np.searchsorted(
                np.asarray(asl[1:NS]), np.arange(ntiles), side="right")

            def fire_ag(s):
                w = int(slice_win[s])
                off = 8 * (asl[s] - wb[w]) * TILE_P
                nc.gpsimd.collective_compute(
                    "AllGather", mybir.AluOpType.bypass,
                    replica_groups=[list(range(NC_CORES))],
                    ins=[hs_shard[s].opt()],
                    outs=[hs_win[w][off:off + NC_CORES * sl_rows[s], :].opt()])

            PRE = 3  # tiles whose gathers are issued before the main loop
            g_pend = {}
            g_done = {}
            # SWDGE queue must advance in lockstep with issue order: the tile
            # scheduler hands out DMASW sem lanes round-robin per SWDGE
            # instruction, and each sem is locked to one queue — a strict
            # global cycle keeps lane<->queue consistent.
            gq = [0]

            def gather_win(t, w, G):
                cnt = int(cW[t, w])
                if cnt == 0:
                    return
                o_rel = int(cW[t, :w].sum())
                cs = int(cs2[t]) + o_rel
                q = gq[0] % N_QUEUES
                gq[0] += 1
                nc.gpsimd.dma_gather(
                    G[:, o_rel:o_rel + cnt, :], hs_win[w][:, :],
                    idx_sb[:, cs * 8:(cs + cnt) * 8],
                    cnt * 128, cnt * 128, F,
                    single_packet=(cnt * 128 <= 128),
                    queue_num=q)

            def agg_matmuls(aggp, P, G, t, nch):
                """scatter-add: self term (fp16) + fp8 DoubleRow chunk pairs."""
                nc.tensor.matmul(aggp[:], id16_sb[:], self_sb[:, t, :],
                                 start=True, stop=(nch == 0))
                c = 0
                while c < nch:
                    if c + 2 <= nch:
                        nc.tensor.matmul(aggp[:], P[:, c:c + 2, :],
                                         G[:, c:c + 2, :], start=False,
                                         stop=(c + 2 == nch), perf_mode=DR)
                        c += 2
                    else:
                        nc.tensor.matmul(aggp[:], P[:, c, :], G[:, c, :],
                                         start=False, stop=True)
                        c += 1

            for layer in range(2):
                w_sb = w1_sb if layer == 0 else w2_sb
                toff_sb = toff1_sb if layer == 0 else toff2_sb

                def build_p(t):
                    """one-hot matrices for tile t (VectorE), built one tile
                    ahead so the PE never waits on them."""
                    if layer == 0:
                        nch, cs = int(nch1[t]), int(cs1[t])
                    else:
                        nch, cs = int(nch2[t]), int(cs2[t])
                    if not nch:
                        return None
                    P = ppool.tile([128, nch, 128], f8, tag="P")
                    nc.vector.tensor_tensor(
                        P[:],
                        iota_sb[:].unsqueeze(1).broadcast_to([128, nch, 128]),
                        toff_sb[:, cs:cs + nch].unsqueeze(2).broadcast_to(
                            [128, nch, 128]),
                        eq)
                    return P

                def stage_a(t, P):
                    """gather/stream G, scatter-add the incoming messages +
                    self term into PSUM, copy to SBUF (ScalarE)."""
                    if layer == 0:
                        nch, cs = int(nch1[t]), int(cs1[t])
                        G = gpool.tile([128, max(nch, 1), F], f8, tag="G")
                        if nch:
                            nc.sync.dma_start(
                                G[:, 0:nch, :],
                                g1_d[:, cs * F:(cs + nch) * F].rearrange(
                                    "p (c f) -> p c f", f=F))
                    else:
                        nch = int(nch2[t])
                        if t in g_pend:
                            G = g_pend.pop(t)
                            done = g_done.pop(t)
                        else:
                            G = gpool.tile([128, max(nch, 1), F], f8,
                                           tag="G")
                            done = ()
                        # rotate the window issue order per tile: with
                        # W == N_QUEUES a fixed order would pin window w
                        # to queue w permanently (the global queue cycle
                        # advances by W per tile), concentrating the big
                        # window-0 traffic on one queue ring
                        for dw in range(W):
                            w = (dw + t) % W
                            if w not in done:
                                gather_win(t, w, G)
                    # scatter-add (+ self term via identity weights)
                    aggp = psa.tile([128, F], f32, tag="aggp")
                    agg_matmuls(aggp, P, G, t, nch)
                    # PSUM -> SBUF f16 (ScalarE; Dis scaling folded into the
                    # final activation instead)
                    aggc = wpool.tile([128, F], f16, tag="aggc")
                    nc.scalar.activation(aggc[:], aggp[:], AF.Copy)
                    return aggc

                def stage_t(t, aggc):
                    """TensorE transpose of the aggregate + copy out of PSUM."""
                    pT = psb.tile([128, F], f16, tag="pT")
                    for i in range(nf):
                        nc.tensor.transpose(pT[:, 128 * i:128 * (i + 1)],
                                            aggc[:, 128 * i:128 * (i + 1)],
                                            id16_sb[:])
                    aggT = wpool.tile([128, nf, 128], f16, tag="aggT")
                    nc.vector.tensor_copy(
                        aggT[:].rearrange("p a b -> p (a b)"), pT[:])
                    return aggT

                def stage_b(t, aggT):
                    """dense weight matmul + scaled activation + writeback."""
                    zp = psc.tile([128, F], f32, tag="zp")
                    for i in range(nf):
                        nc.tensor.matmul(zp[:], aggT[:, i, :], w_sb[:, i, :],
                                         start=(i == 0), stop=(i == nf - 1))
                    r0, r1 = TILE_P * t, TILE_P * (t + 1)
                    if layer == 0:
                        zin = zp[:]
                        if has_b1:
                            zb = wpool.tile([128, F], f32, tag="zb")
                            nc.vector.tensor_tensor(zb[:], zp[:], b1_sb[:], add)
                            zin = zb[:]
                        # self_sb[t] := dis^2 * relu(z) == dis * relu(dis * z)
                        nc.scalar.activation(self_sb[:, t, :], zin, AF.Relu,
                                             scale=dis2_sb[:, t:t + 1])
                        # fp8 copy of the same activation for the AllGather /
                        # layer-2 gather stream
                        h8 = hpool.tile([128, F], f8, tag="h8")
                        nc.scalar.activation(h8[:], zin, AF.Relu,
                                             scale=dis2_sb[:, t:t + 1])
                        s = int(tile_slice[t])
                        b0 = (t - asl[s]) * TILE_P
                        # hs writes ride the ScalarE HWDGE queue: off the
                        # SWDGE lanes (whose sem rotation the gathers own)
                        # and off the SP ring (so the g1 stream never waits)
                        nc.scalar.dma_start(hs_shard[s][b0:b0 + TILE_P, :],
                                            h8[:])
                        if t in ag_fire:
                            k = ag_fire[t]
                            fire_ag(k)
                            # prefetch the first tiles' gathers once their
                            # window's AllGather slices have all fired: the
                            # non-final windows at the second-to-last fire
                            # (their data landed long ago — no engine-
                            # blocking waits before the final AG fires), the
                            # last window right after the final fire.  A
                            # dedicated pool keeps these allocations out of
                            # the main G ring.
                            if k == NS - 2 and NS >= 2 and \
                                    asl[k + 1] >= ntiles - 1:
                                for tt in range(min(PRE, ntiles)):
                                    nch_t = int(nch2[tt])
                                    G = prepool.tile([128, max(nch_t, 1), F],
                                                     f8, tag="Gpre")
                                    g_pend[tt] = G
                                    g_done[tt] = set()
                                    # only windows whose AllGather landed
                                    # long ago: a not-yet-landed window
                                    # would block the gpsimd engine before
                                    # the final AllGather fires
                                    for w in range(max(W - 3, 0)):
                                        gather_win(tt, w, G)
                                        g_done[tt].add(w)
                            elif k == NS - 1:
                                if not g_pend:
                                    for tt in range(min(PRE, ntiles)):
                                        nch_t = int(nch2[tt])
                                        G = prepool.tile(
                                            [128, max(nch_t, 1), F],
                                            f8, tag="Gpre")
                                        g_pend[tt] = G
                                        g_done[tt] = set()
                                for tt in sorted(g_pend):
                                    for w in range(W):
                                        if w not in g_done[tt]:
                                            gather_win(tt, w, g_pend[tt])
                                            g_done[tt].add(w)
                    else:
                        o_t = wpool.tile([128, F], f16, tag="ot")
                        zin = zp[:]
                        if has_b2:
                            zb = wpool.tile([128, F], f32, tag="zb")
                            nc.vector.tensor_tensor(zb[:], zp[:], b2_sb[:], add)
                            zin = zb[:]
                        # out := dis * z  (SP ring is idle in layer 2)
                        nc.scalar.activation(o_t[:], zin, AF.Copy,
                                             scale=dis_sb[:, t:t + 1])
                        nc.sync.dma_start(out_d[r0:r1, :], o_t[:])

                # 2-deep software pipeline: PE order is aggp(t), T(t-1),
                # zp(t-2) so the TensorE stream never stalls on the
                # cross-engine transpose round-trip; P built one tile ahead
                p_next = build_p(0)
                aggc_q, aggt_q = {}, {}
                for t in range(ntiles + 2):
                    if t < ntiles:
                        P_cur = p_next
                        p_next = build_p(t + 1) if t + 1 < ntiles else None
                        aggc_q[t] = stage_a(t, P_cur)
                    if 1 <= t <= ntiles:
                        aggt_q[t - 1] = stage_t(t - 1, aggc_q.pop(t - 1))
                    if t >= 2:
                        stage_b(t - 2, aggt_q.pop(t - 2))

    nc.compile()
    return nc


def kernel(x, edge_index, W1, b1, W2, b2):
    x = np.asarray(x, dtype=np.float32)
    W1 = np.asarray(W1, dtype=np.float32)
    W2 = np.asarray(W2, dtype=np.float32)
    b1 = np.asarray(b1, dtype=np.float32)
    b2 = np.asarray(b2, dtype=np.float32)
    meta = _prep_host(x, edge_index)

    has_b1 = bool(np.any(b1))
    has_b2 = bool(np.any(b2))
    nc = _build_program(meta, has_b1, has_b2)

    in_maps = []
    for p in range(NC_CORES):
        m = {
            "g1": meta["g1"][p],
            "xsl": meta["xsl"][p],
            "idx": meta["idx2"][p],
            "toff1": meta["toff1"][p],
            "toff2": meta["toff2"][p],
            "dis": meta["dis_tiles"][p],
            "dis2": meta["dis2_tiles"][p],
            "w1": W1.astype(np.float16), "w2": W2.astype(np.float16),
            "id16": np.eye(128, dtype=np.float16),
            "iota": np.tile(np.arange(128, dtype=np.float16), (128, 1)),
        }
        if has_b1:
            m["b1r"] = np.tile(b1, (128, 1)).astype(np.float32)
        if has_b2:
            m["b2r"] = np.tile(b2, (128, 1)).astype(np.float32)
        in_maps.append(m)

    if os.environ.get("GNN_SIM", "0") == "1":
        from concourse.bass_interp import MultiCoreSim
        sim = MultiCoreSim(nc, num_cores=NC_CORES, trace=False)
        cores = list(sim.cores.values())
        for p, core in enumerate(cores):
            for k, v in in_maps[p].items():
                core.tensor(k)[:] = v
        sim.simulate(check_with_hw=False)
        shards = [cores[p].tensor("out").copy() for p in range(NC_CORES)]
    else:
        from concourse import bass_utils
        trace = os.environ.get("GNN_TRACE", "0") == "1"
        res = bass_utils.run_bass_kernel_spmd(
            nc, in_maps, core_ids=list(range(NC_CORES)), trace=trace)
        if trace and res.exec_time_ns is not None:
            print(f"HW exec time: {res.exec_time_ns} ns")
        kernel.last_results = res
        shards = [res.results[p]["out"] for p in range(NC_CORES)]

    npc = meta["npc"]
    out = np.concatenate([s[:npc] for s in shards], axis=0)
    return out.astype(np.float32)


# revision 11
# speedup vs baseline: 1.0248x; 1.0104x over previous
"""Trainium2 Bass kernel: 2-layer GCN (GCNConv -> ReLU -> GCNConv).

Math:  S = D^-1/2 (A + I) D^-1/2  (A from edge_index, self-loops appended)
       out = S @ relu(S @ x @ W1 + b1) @ W2 + b2
Using linearity, aggregate-then-matmul per layer with u = Dis*x:
       agg1 = A' @ u + u            (A' = adjacency without self-loops)
       h~   = Dis^2 * relu(agg1 @ W1 (+ b1))   (= Dis * h1, stored fp16)
       agg2 = A' @ h~ + h~
       out  = Dis * (agg2 @ W2) (+ b2)
The Dis row-scaling commutes with the dense matmul, so it is folded into a
single ScalarE activation per tile (scale = dis^2 resp. dis, func=relu/copy).

Distribution: nodes sharded over 8 NeuronCores.  Per layer, each core
scatter-adds incoming-edge source rows per 128-target tile with TensorE
matmuls against one-hot matrices (built on VectorE via is_equal vs iota, one
tile ahead of use), transposes via TensorE and applies the dense weight
matmul in fp16, all on a 2-deep software pipeline (PE order: aggregate(t),
transpose(t-1), dense(t-2)) so the TensorE stream never stalls.

Precision: the gathered/streamed neighbor rows, the one-hot matrices and the
AllGather payload are fp8e4 (self term, weights and dense path stay fp16;
measured HW rel-err 1.8e-2 vs the 2e-2 gate).  The scatter-add runs two fp8
chunks per PE pass with MatmulPerfMode.DoubleRow (2x fp8 throughput), so
both HBM traffic and aggregation PE time halve vs fp16.
  Layer 1: gather indices are static and the source data (fp8 Dis*x) is a
  kernel input, so the HOST pre-expands the gathered stream into edge order;
  the device streams it sequentially over HWDGE at full HBM bandwidth.
  Layer 2: fp8 activations are AllGathered into TWO window tensors (each
  int16-indexable: 8*window_rows <= 32767) so each tile needs only two
  dma_gather calls, but the collective itself is cut into ~8 small SLICES
  (separate input shard tensors, sliced output APs) fired as soon as each
  slice's tiles are computed — the CC stream pipelines behind layer-1
  compute and only a tiny final slice is exposed at the layer boundary.
  Gather descriptor generation cycles the 4 SWDGE queues in strict issue
  order (the tile scheduler's DMASW sem lanes are queue-locked); gather
  indices are pre-sorted for HBM locality.
"""

import os
import numpy as np
import ml_dtypes

NC_CORES = 8
TILE_P = 128
N_QUEUES = 4
WIN_TILES = 31  # 8 * 31 * 128 = 31744 <= 32767 (int16 gather index limit)
FP8 = ml_dtypes.float8_e4m3


def _round_up(v, m):
    return (v + m - 1) // m * m


def _windows(ntiles):
    """Gather windows (int16-indexable) as tile bounds.  Each window is
    also one AllGather (a Shared tensor has a single writer), so sizes
    taper: big early windows fire mid-layer-1 and their collectives hide
    behind compute; the late windows are small (quick collectives); the
    final 1-tile window keeps the exposed boundary latency tiny."""
    if ntiles <= 2:
        return [0, ntiles]
    b = [0]
    main = ntiles - 1  # reserve the 1-tile final window
    # even-ish thirds up to 18 tiles each: the first window's collective
    # fires early (hides behind layer 1), later ones shrink
    while main - b[-1] > 18:
        b.append(min(b[-1] + 18, main))
    rem = main - b[-1]
    if rem > 14:
        b.append(b[-1] + (rem * 2) // 3)
    b.append(main)
    b.append(ntiles)
    return sorted(set(b))


def _prep_host(x, edge_index):
    """Partition + pad the graph; build per-core stream/gather metadata."""
    x = np.asarray(x, dtype=np.float32)
    edge_index = np.asarray(edge_index)
    N, F = x.shape
    assert N % NC_CORES == 0, (N, NC_CORES)
    npc = N // NC_CORES
    npc_pad = _round_up(npc, TILE_P)
    ntiles = npc_pad // TILE_P
    n_pad = NC_CORES * npc_pad

    loops = np.arange(N, dtype=np.int64)
    # edges WITHOUT self-loops (self term handled on-device)
    row = edge_index[0].astype(np.int64)
    col = edge_index[1].astype(np.int64)
    # degree WITH self-loops (as the reference computes it)
    deg = np.bincount(np.concatenate([col, loops]), minlength=N).astype(np.float64)
    dis = np.where(deg > 0, 1.0 / np.sqrt(deg), 0.0).astype(np.float32)

    src_pad = (row // npc) * npc_pad + (row % npc)
    tgt_core = (col // npc).astype(np.int64)
    tgt_loc = col % npc
    tile_of = tgt_loc // TILE_P
    toff_of = (tgt_loc % TILE_P).astype(np.float32)

    # ---- layer 1: host-expanded stream, grouped by (core, tile) ----
    key1 = tgt_core * ntiles + tile_of
    cnt1 = np.bincount(key1, minlength=NC_CORES * ntiles).reshape(
        NC_CORES, ntiles)
    C1 = _round_up(cnt1.max(axis=0), TILE_P)  # padded counts [ntiles]
    tot1 = int(C1.sum())
    totch1 = tot1 // TILE_P
    nch1 = (C1 // TILE_P).astype(np.int64)
    cs1 = np.zeros(ntiles, dtype=np.int64)
    np.cumsum(nch1[:-1], out=cs1[1:])

    o1 = np.argsort(key1, kind="stable")
    src1_s, toff1_s = src_pad[o1], toff_of[o1]
    g1start = np.zeros(NC_CORES * ntiles + 1, dtype=np.int64)
    np.cumsum(cnt1.reshape(-1), out=g1start[1:])

    # ---- layer 2: dma_gather over W windows (= AllGather pieces) ----
    wb = _windows(ntiles)          # window bounds (tiles)
    W = len(wb) - 1
    asl = wb                       # AG slices coincide with windows
    NS = len(asl) - 1
    win_rows = [(wb[w + 1] - wb[w]) * TILE_P for w in range(W)]
    for r in win_rows:
        assert NC_CORES * r <= 32767, (win_rows,)
    slice_win = np.searchsorted(np.asarray(wb[1:W]),
                                np.asarray(asl[:-1]), side="right")

    src_core = row // npc
    src_loc = row % npc
    src_tile = src_loc // TILE_P
    win = np.searchsorted(np.asarray(wb[1:W]), src_tile, side="right")
    sl = np.searchsorted(np.asarray(asl[1:NS]), src_tile, side="right")
    asl_arr = np.asarray(asl)
    wb_arr = np.asarray(wb)
    rows_s = (asl_arr[sl + 1] - asl_arr[sl]) * TILE_P
    # global row inside the window tensor: slice block offset + core-major
    # position inside the slice
    gidx = (8 * (asl_arr[sl] - wb_arr[win]) * TILE_P
            + src_core * rows_s + (src_loc - asl_arr[sl] * TILE_P))

    key2 = (tgt_core * ntiles + tile_of) * W + win
    cnt2 = np.bincount(key2, minlength=NC_CORES * ntiles * W).reshape(
        NC_CORES, ntiles, W)
    C2 = cnt2.max(axis=0)
    C2 = np.where(C2 > 0, _round_up(C2, TILE_P), 0)  # [ntiles, W]
    tot2 = int(C2.sum())
    cW = (C2 // TILE_P).astype(np.int64)  # chunks per (tile, window)
    nch2 = cW.sum(axis=1)
    cs2 = np.zeros(ntiles, dtype=np.int64)
    np.cumsum(nch2[:-1], out=cs2[1:])

    # secondary sort by source index: the one-hot P absorbs any within-group
    # permutation, and ascending gather addresses improve HBM locality
    o2 = np.lexsort((gidx, key2))
    src2_s, toff2_s = gidx[o2], toff_of[o2]
    g2start = np.zeros(NC_CORES * ntiles * W + 1, dtype=np.int64)
    np.cumsum(cnt2.reshape(-1), out=g2start[1:])

    # padded Dis*x: fp8 copy feeds the edge streams, fp16 the self term
    xs32 = dis[:, None] * x
    xs_pad8 = np.zeros((NC_CORES, npc_pad, F), dtype=FP8)
    xs_pad8[:, :npc] = xs32.reshape(NC_CORES, npc, F).astype(FP8)
    xs_pad16 = np.zeros((NC_CORES, npc_pad, F), dtype=np.float16)
    xs_pad16[:, :npc] = xs32.reshape(NC_CORES, npc, F).astype(np.float16)
    xsl = np.ascontiguousarray(
        xs_pad16.reshape(NC_CORES, ntiles, TILE_P, F).transpose(0, 2, 1, 3))
    xs_flat8 = xs_pad8.reshape(n_pad, F)

    g1 = np.zeros((NC_CORES, 128, totch1, F), dtype=FP8)
    toff1 = np.full((NC_CORES, tot1), -1.0, dtype=np.float32)
    idx2 = np.zeros((NC_CORES, max(tot2, 16)), dtype=np.int16)
    toff2 = np.full((NC_CORES, max(tot2, TILE_P)), -1.0, dtype=np.float32)
    for p in range(NC_CORES):
        off = 0
        for t in range(ntiles):
            g = p * ntiles + t
            a, b = g1start[g], g1start[g + 1]
            n = b - a
            blk = g1[p, :, cs1[t]:cs1[t] + nch1[t], :]
            j = np.arange(n)
            # stream row j -> partition j%128, chunk j//128
            blk[j % 128, j // 128] = xs_flat8[src1_s[a:b]]
            toff1[p, off:off + n] = toff1_s[a:b]
            off += C1[t]
        off = 0
        for t in range(ntiles):
            for h in range(W):
                c = int(C2[t, h])
                if c == 0:
                    continue
                g = (p * ntiles + t) * W + h
                a, b = g2start[g], g2start[g + 1]
                n = b - a
                s = src2_s[a:b]
                assert n <= c and (s >= 0).all() and (s < 32767).all()
                idx2[p, off:off + n] = s.astype(np.int16)
                toff2[p, off:off + n] = toff2_s[a:b]
                off += c

    tot2c = max(tot2, 16)
    idx2_w = np.ascontiguousarray(
        np.tile(idx2.reshape(NC_CORES, tot2c // 16, 16).transpose(0, 2, 1),
                (1, 8, 1)))
    toff1_w = np.ascontiguousarray(
        toff1.reshape(NC_CORES, totch1, TILE_P).transpose(0, 2, 1)).astype(
            np.float16)
    tot2t = max(tot2, TILE_P)
    toff2_w = np.ascontiguousarray(
        toff2.reshape(NC_CORES, tot2t // TILE_P, TILE_P).transpose(0, 2, 1)
    ).astype(np.float16)

    dis_pad = np.zeros((NC_CORES, npc_pad), dtype=np.float32)
    dis_pad[:, :npc] = dis.reshape(NC_CORES, npc)
    dis_tiles = np.ascontiguousarray(
        dis_pad.reshape(NC_CORES, ntiles, TILE_P).transpose(0, 2, 1))
    dis2_tiles = np.ascontiguousarray(dis_tiles * dis_tiles)

    return dict(
        N=N, F=F, npc=npc, npc_pad=npc_pad, ntiles=ntiles, n_pad=n_pad,
        wb=wb, W=W, asl=asl, NS=NS, win_rows=win_rows, slice_win=slice_win,
        nch1=nch1, cs1=cs1, totch1=totch1,
        cW=cW, nch2=nch2, cs2=cs2, tot2=tot2,
        g1=g1.reshape(NC_CORES, 128, totch1 * F),
        xsl=xsl.reshape(NC_CORES, 128, ntiles * F),
        idx2=idx2_w, toff1=toff1_w, toff2=toff2_w,
        dis_tiles=dis_tiles, dis2_tiles=dis2_tiles,
    )


def _build_program(meta, has_b1, has_b2):
    import concourse.bacc as bacc
    import concourse.tile as tile
    from concourse import mybir

    F = meta["F"]
    ntiles = meta["ntiles"]
    npc_pad = meta["npc_pad"]
    wb, W, asl, NS = meta["wb"], meta["W"], meta["asl"], meta["NS"]
    win_rows, slice_win = meta["win_rows"], meta["slice_win"]
    nch1, cs1, totch1 = meta["nch1"], meta["cs1"], meta["totch1"]
    cW, nch2, cs2 = meta["cW"], meta["nch2"], meta["cs2"]
    totw2 = max(meta["tot2"], 16) // 16
    totch2 = max(meta["tot2"], TILE_P) // TILE_P
    nf = F // TILE_P
    f32, f16, i16 = mybir.dt.float32, mybir.dt.float16, mybir.dt.int16
    f8 = mybir.dt.float8e4
    AF = mybir.ActivationFunctionType
    DR = mybir.MatmulPerfMode.DoubleRow

    nc = bacc.Bacc("TRN2", target_bir_lowering=False, debug=False,
                   num_devices=NC_CORES, num_swdge_queues=N_QUEUES)

    g1_d = nc.dram_tensor("g1", [128, totch1 * F], f8, kind="ExternalInput")
    xsl_d = nc.dram_tensor("xsl", [128, ntiles * F], f16, kind="ExternalInput")
    idx_d = nc.dram_tensor("idx", [128, totw2], i16, kind="ExternalInput")
    toff1_d = nc.dram_tensor("toff1", [128, totch1], f16, kind="ExternalInput")
    toff2_d = nc.dram_tensor("toff2", [128, totch2], f16, kind="ExternalInput")
    dis_d = nc.dram_tensor("dis", [128, ntiles], f32, kind="ExternalInput")
    dis2_d = nc.dram_tensor("dis2", [128, ntiles], f32, kind="ExternalInput")
    w1_d = nc.dram_tensor("w1", [F, F], f16, kind="ExternalInput")
    w2_d = nc.dram_tensor("w2", [F, F], f16, kind="ExternalInput")
    id16_d = nc.dram_tensor("id16", [128, 128], f16, kind="ExternalInput")
    iota_d = nc.dram_tensor("iota", [128, 128], f16, kind="ExternalInput")
    if has_b1:
        b1_d = nc.dram_tensor("b1r", [128, F], f32, kind="ExternalInput")
    if has_b2:
        b2_d = nc.dram_tensor("b2r", [128, F], f32, kind="ExternalInput")
    out_d = nc.dram_tensor("out", [npc_pad, F], f16, kind="ExternalOutput")

    eq, add = mybir.AluOpType.is_equal, mybir.AluOpType.add

    with tile.TileContext(nc) as tc:
        with (
            tc.tile_pool(name="const", bufs=1) as cpool,
            tc.tile_pool(name="gbuf", bufs=4) as gpool,
            tc.tile_pool(name="gpre", bufs=3) as prepool,
            tc.tile_pool(name="pbuf", bufs=3) as ppool,
            tc.tile_pool(name="work", bufs=4) as wpool,
            tc.tile_pool(name="h8buf", bufs=3) as hpool,
            tc.tile_pool(name="psA", bufs=3, space="PSUM") as psa,
            tc.tile_pool(name="psB", bufs=2, space="PSUM") as psb,
            tc.tile_pool(name="psC", bufs=3, space="PSUM") as psc,
            tc.tile_pool(name="dram", bufs=1, space="DRAM") as dpool,
        ):
            idx_sb = cpool.tile([128, totw2], i16)
            nc.sync.dma_start(idx_sb[:], idx_d[:, :])
            toff1_sb = cpool.tile([128, totch1], f16)
            nc.sync.dma_start(toff1_sb[:], toff1_d[:, :])
            toff2_sb = cpool.tile([128, totch2], f16)
            nc.sync.dma_start(toff2_sb[:], toff2_d[:, :])
            dis_sb = cpool.tile([128, ntiles], f32)
            nc.sync.dma_start(dis_sb[:], dis_d[:, :])
            dis2_sb = cpool.tile([128, ntiles], f32)
            nc.sync.dma_start(dis2_sb[:], dis2_d[:, :])
            id16_sb = cpool.tile([128, 128], f16)
            nc.sync.dma_start(id16_sb[:], id16_d[:, :])
            iota_sb = cpool.tile([128, 128], f16)
            nc.sync.dma_start(iota_sb[:], iota_d[:, :])
            w1_sb = cpool.tile([128, nf, F], f16)
            w2_sb = cpool.tile([128, nf, F], f16)
            for i in range(nf):
                nc.sync.dma_start(w1_sb[:, i, :], w1_d[128 * i:128 * (i + 1), :])
                nc.sync.dma_start(w2_sb[:, i, :], w2_d[128 * i:128 * (i + 1), :])
            if has_b1:
                b1_sb = cpool.tile([128, F], f32)
                nc.sync.dma_start(b1_sb[:], b1_d[:, :])
            if has_b2:
                b2_sb = cpool.tile([128, F], f32)
                nc.sync.dma_start(b2_sb[:], b2_d[:, :])

            # local shard, fp16: holds Dis*x during layer 1, then Dis*h1
            self_sb = cpool.tile([128, ntiles, F], f16)
            nc.sync.dma_start(
                self_sb[:], xsl_d[:, :].rearrange("p (t f) -> p t f", f=F))

            # per-AG-slice input shards (separate tensors: a slice is only
            # written by its own tiles, so firing its AllGather never
            # serializes against later hs writes), per-WINDOW gather sources
            sl_rows = [(asl[s + 1] - asl[s]) * TILE_P for s in range(NS)]
            hs_shard = [dpool.tile([sl_rows[s], F], f8, name=f"hs_shard{s}")
                        for s in range(NS)]
            hs_win = [dpool.tile([NC_CORES * win_rows[w], F], f8,
                                 addr_space="Shared", name=f"hs_win{w}")
                      for w in range(W)]
            # AG slice -> tile index whose stage_b fires it
            ag_fire = {asl[s + 1] - 1: s for s in range(NS)}
            # tile -> AG slice
            tile_slice = np.searchsorted(
                np.asarray(asl[1:NS]), np.arange(ntiles), side="right")

            def fire_ag(s):
                w = int(slice_win[s])
                off = 8 * (asl[s] - wb[w]) * TILE_P
                nc.gpsimd.collective_compute(
                    "AllGather", mybir.AluOpType.bypass,
                    replica_groups=[list(range(NC_CORES))],
                    ins=[hs_shard[s].opt()],
                    outs=[hs_win[w][off:off + NC_CORES * sl_rows[s], :].opt()])

            PRE = 3  # tiles whose gathers are issued before the main loop
            g_pend = {}
            g_done = {}
            # SWDGE queue must advance in lockstep with issue order: the tile
            # scheduler hands out DMASW sem lanes round-robin per SWDGE
            # instruction, and each sem is locked to one queue — a strict
            # global cycle keeps lane<->queue consistent.
            gq = [0]

            def gather_win(t, w, G):
                cnt = int(cW[t, w])
                if cnt == 0:
                    return
                o_rel = int(cW[t, :w].sum())
                cs = int(cs2[t]) + o_rel
                q = gq[0] % N_QUEUES
                gq[0] += 1
                nc.gpsimd.dma_gather(
                    G[:, o_rel:o_rel + cnt, :], hs_win[w][:, :],
                    idx_sb[:, cs * 8:(cs + cnt) * 8],
                    cnt * 128, cnt * 128, F,
                    single_packet=(cnt * 128 <= 128),
                    queue_num=q)

            def agg_matmuls(aggp, P, G, t, nch):
                """scatter-add: self term (fp16) + fp8 DoubleRow chunk pairs."""
                nc.tensor.matmul(aggp[:], id16_sb[:], self_sb[:, t, :],
                                 start=True, stop=(nch == 0))
                c = 0
                while c < nch:
                    if c + 2 <= nch:
                        nc.tensor.matmul(aggp[:], P[:, c:c + 2, :],
                                         G[:, c:c + 2, :], start=False,
                                         stop=(c + 2 == nch), perf_mode=DR)
                        c += 2
                    else:
                        nc.tensor.matmul(aggp[:], P[:, c, :], G[:, c, :],
                                         start=False, stop=True)
                        c += 1

            for layer in range(2):
                w_sb = w1_sb if layer == 0 else w2_sb
                toff_sb = toff1_sb if layer == 0 else toff2_sb

                def build_p(t):
                    """one-hot matrices for tile t (VectorE), built one tile
                    ahead so the PE never waits on them."""
                    if layer == 0:
                        nch, cs = int(nch1[t]), int(cs1[t])
                    else:
                        nch, cs = int(nch2[t]), int(cs2[t])
                    if not nch:
                        return None
                    P = ppool.tile([128, nch, 128], f8, tag="P")
                    nc.vector.tensor_tensor(
                        P[:],
                        iota_sb[:].unsqueeze(1).broadcast_to([128, nch, 128]),
                        toff_sb[:, cs:cs + nch].unsqueeze(2).broadcast_to(
                            [128, nch, 128]),
                        eq)
                    return P

                def stage_a(t, P):
                    """gather/stream G, scatter-add the incoming messages +
                    self term into PSUM, copy to SBUF (ScalarE)."""
                    if layer == 0:
                        nch, cs = int(nch1[t]), int(cs1[t])
                        G = gpool.tile([128, max(nch, 1), F], f8, tag="G")
                        if nch:
                            nc.sync.dma_start(
                                G[:, 0:nch, :],
                                g1_d[:, cs * F:(cs + nch) * F].rearrange(
                                    "p (c f) -> p c f", f=F))
                    else:
                        nch = int(nch2[t])
                        if t in g_pend:
                            G = g_pend.pop(t)
                            done = g_done.pop(t)
                        else:
                            G = gpool.tile([128, max(nch, 1), F], f8,
                                           tag="G")
                            done = ()
                        # rotate the window issue order per tile: with
                        # W == N_QUEUES a fixed order would pin window w
                        # to queue w permanently (the global queue cycle
                        # advances by W per tile), concentrating the big
                        # window-0 traffic on one queue ring
                        for dw in range(W):
                            w = (dw + t) % W
                            if w not in done:
                                gather_win(t, w, G)
                    # scatter-add (+ self term via identity weights)
                    aggp = psa.tile([128, F], f32, tag="aggp")
                    agg_matmuls(aggp, P, G, t, nch)
                    # PSUM -> SBUF f16 (ScalarE; Dis scaling folded into the
                    # final activation instead)
                    aggc = wpool.tile([128, F], f16, tag="aggc")
                    nc.scalar.activation(aggc[:], aggp[:], AF.Copy)
                    return aggc

                def stage_t(t, aggc):
                    """TensorE transpose of the aggregate + copy out of PSUM."""
                    pT = psb.tile([128, F], f16, tag="pT")
                    for i in range(nf):
                        nc.tensor.transpose(pT[:, 128 * i:128 * (i + 1)],
                                            aggc[:, 128 * i:128 * (i + 1)],
                                            id16_sb[:])
                    aggT = wpool.tile([128, nf, 128], f16, tag="aggT")
                    nc.vector.tensor_copy(
                        aggT[:].rearrange("p a b -> p (a b)"), pT[:])
                    return aggT

                def stage_b(t, aggT):
                    """dense weight matmul + scaled activation + writeback."""
                    zp = psc.tile([128, F], f32, tag="zp")
                    for i in range(nf):
                        nc.tensor.matmul(zp[:], aggT[:, i, :], w_sb[:, i, :],
                                         start=(i == 0), stop=(i == nf - 1))
                    r0, r1 = TILE_P * t, TILE_P * (t + 1)
                    if layer == 0:
                        zin = zp[:]
                        if has_b1:
                            zb = wpool.tile([128, F], f32, tag="zb")
                            nc.vector.tensor_tensor(zb[:], zp[:], b1_sb[:], add)
                            zin = zb[:]
                        # self_sb[t] := dis^2 * relu(z) == dis * relu(dis * z)
                        nc.scalar.activation(self_sb[:, t, :], zin, AF.Relu,
                                             scale=dis2_sb[:, t:t + 1])
                        # fp8 copy of the same activation for the AllGather /
                        # layer-2 gather stream
                        h8 = hpool.tile([128, F], f8, tag="h8")
                        nc.scalar.activation(h8[:], zin, AF.Relu,
                                             scale=dis2_sb[:, t:t + 1])
                        s = int(tile_slice[t])
                        b0 = (t - asl[s]) * TILE_P
                        # hs writes ride the ScalarE HWDGE queue: off the
                        # SWDGE lanes (whose sem rotation the gathers own)
                        # and off the SP ring (so the g1 stream never waits)
                        nc.scalar.dma_start(hs_shard[s][b0:b0 + TILE_P, :],
                                            h8[:])
                        if t in ag_fire:
                            k = ag_fire[t]
                            fire_ag(k)
                            # prefetch the first tiles' gathers once their
                            # window's AllGather slices have all fired: the
                            # non-final windows at the second-to-last fire
                            # (their data landed long ago — no engine-
                            # blocking waits before the final AG fires), the
                            # last window right after the final fire.  A
                            # dedicated pool keeps these allocations out of
                            # the main G ring.
                            if k == NS - 2 and NS >= 2 and \
                                    asl[k + 1] >= ntiles - 1:
                                for tt in range(min(PRE, ntiles)):
                                    nch_t = int(nch2[tt])
                                    G = prepool.tile([128, max(nch_t, 1), F],
                                                     f8, tag="Gpre")
                                    g_pend[tt] = G
                                    g_done[tt] = set()
                                    # only windows whose AllGather landed
                                    # long ago: a not-yet-landed window
                                    # would block the gpsimd engine before
                                    # the final AllGather fires
                                    for w in range(max(W - 3, 0)):
                                        gather_win(tt, w, G)
                                        g_done[tt].add(w)
                            elif k == NS - 1:
                                if not g_pend:
                                    for tt in range(min(PRE, ntiles)):
                                        nch_t = int(nch2[tt])
                                        G = prepool.tile(
                                            [128, max(nch_t, 1), F],
                                            f8, tag="Gpre")
                                        g_pend[tt] = G
                                        g_done[tt] = set()
                                for tt in sorted(g_pend):
                                    for w in range(W):
                                        if w not in g_done[tt]:
                                            gather_win(tt, w, g_pend[tt])
                                            g_done[tt].add(w)
                    else:
                        o_t = wpool.tile([128, F], f16, tag="ot")
                        zin = zp[:]
                        if has_b2:
                            zb = wpool.tile([128, F], f32, tag="zb")
                            nc.vector.tensor_tensor(zb[:], zp[:], b2_sb[:], add)
                            zin = zb[:]
                        # out := dis * z  (SP ring is idle in layer 2)
                        nc.scalar.activation(o_t[:], zin, AF.Copy,
                                             scale=dis_sb[:, t:t + 1])
                        nc.sync.dma_start(out_d[r0:r1, :], o_t[:])

                # 2-deep software pipeline: PE order is aggp(t), T(t-1),
                # zp(t-2) so the TensorE stream never stalls on the
                # cross-engine transpose round-trip; P built one tile ahead
                p_next = build_p(0)
                aggc_q, aggt_q = {}, {}
                for t in range(ntiles + 2):
                    if t < ntiles:
                        P_cur = p_next
                        p_next = build_p(t + 1) if t + 1 < ntiles else None
                        aggc_q[t] = stage_a(t, P_cur)
                    if 1 <= t <= ntiles:
                        aggt_q[t - 1] = stage_t(t - 1, aggc_q.pop(t - 1))
                    if t >= 2:
                        stage_b(t - 2, aggt_q.pop(t - 2))

    nc.compile()
    return nc


def kernel(x, edge_index, W1, b1, W2, b2):
    x = np.asarray(x, dtype=np.float32)
    W1 = np.asarray(W1, dtype=np.float32)
    W2 = np.asarray(W2, dtype=np.float32)
    b1 = np.asarray(b1, dtype=np.float32)
    b2 = np.asarray(b2, dtype=np.float32)
    meta = _prep_host(x, edge_index)

    has_b1 = bool(np.any(b1))
    has_b2 = bool(np.any(b2))
    nc = _build_program(meta, has_b1, has_b2)

    in_maps = []
    for p in range(NC_CORES):
        m = {
            "g1": meta["g1"][p],
            "xsl": meta["xsl"][p],
            "idx": meta["idx2"][p],
            "toff1": meta["toff1"][p],
            "toff2": meta["toff2"][p],
            "dis": meta["dis_tiles"][p],
            "dis2": meta["dis2_tiles"][p],
            "w1": W1.astype(np.float16), "w2": W2.astype(np.float16),
            "id16": np.eye(128, dtype=np.float16),
            "iota": np.tile(np.arange(128, dtype=np.float16), (128, 1)),
        }
        if has_b1:
            m["b1r"] = np.tile(b1, (128, 1)).astype(np.float32)
        if has_b2:
            m["b2r"] = np.tile(b2, (128, 1)).astype(np.float32)
        in_maps.append(m)

    if os.environ.get("GNN_SIM", "0") == "1":
        from concourse.bass_interp import MultiCoreSim
        sim = MultiCoreSim(nc, num_cores=NC_CORES, trace=False)
        cores = list(sim.cores.values())
        for p, core in enumerate(cores):
            for k, v in in_maps[p].items():
                core.tensor(k)[:] = v
        sim.simulate(check_with_hw=False)
        shards = [cores[p].tensor("out").copy() for p in range(NC_CORES)]
    else:
        from concourse import bass_utils
        trace = os.environ.get("GNN_TRACE", "0") == "1"
        res = bass_utils.run_bass_kernel_spmd(
            nc, in_maps, core_ids=list(range(NC_CORES)), trace=trace)
        if trace and res.exec_time_ns is not None:
            print(f"HW exec time: {res.exec_time_ns} ns")
        kernel.last_results = res
        shards = [res.results[p]["out"] for p in range(NC_CORES)]

    npc = meta["npc"]
    out = np.concatenate([s[:npc] for s in shards], axis=0)
    return out.astype(np.float32)


# revision 12
# speedup vs baseline: 1.0398x; 1.0147x over previous
"""Trainium2 Bass kernel: 2-layer GCN (GCNConv -> ReLU -> GCNConv).

Math:  S = D^-1/2 (A + I) D^-1/2  (A from edge_index, self-loops appended)
       out = S @ relu(S @ x @ W1 + b1) @ W2 + b2
Using linearity, aggregate-then-matmul per layer with u = Dis*x:
       agg1 = A' @ u + u            (A' = adjacency without self-loops)
       h~   = Dis^2 * relu(agg1 @ W1 (+ b1))   (= Dis * h1, stored fp16)
       agg2 = A' @ h~ + h~
       out  = Dis * (agg2 @ W2) (+ b2)
The Dis row-scaling commutes with the dense matmul, so it is folded into a
single ScalarE activation per tile (scale = dis^2 resp. dis, func=relu/copy).

Distribution: nodes sharded over 8 NeuronCores.  Per layer, each core
scatter-adds incoming-edge source rows per 128-target tile with TensorE
matmuls against one-hot matrices (built on VectorE via is_equal vs iota, one
tile ahead of use), transposes via TensorE and applies the dense weight
matmul in fp16, all on a 2-deep software pipeline (PE order: aggregate(t),
transpose(t-1), dense(t-2)) so the TensorE stream never stalls.

Precision: the gathered/streamed neighbor rows, the one-hot matrices and the
AllGather payload are fp8e4 (self term, weights and dense path stay fp16;
measured HW rel-err 1.8e-2 vs the 2e-2 gate).  The scatter-add runs two fp8
chunks per PE pass with MatmulPerfMode.DoubleRow (2x fp8 throughput), so
both HBM traffic and aggregation PE time halve vs fp16.
  Layer 1: gather indices are static and the source data (fp8 Dis*x) is a
  kernel input, so the HOST pre-expands the gathered stream into edge order;
  the device streams it sequentially over HWDGE at full HBM bandwidth.
  Layer 2: fp8 activations are AllGathered into TWO window tensors (each
  int16-indexable: 8*window_rows <= 32767) so each tile needs only two
  dma_gather calls, but the collective itself is cut into ~8 small SLICES
  (separate input shard tensors, sliced output APs) fired as soon as each
  slice's tiles are computed — the CC stream pipelines behind layer-1
  compute and only a tiny final slice is exposed at the layer boundary.
  Gather descriptor generation cycles the 4 SWDGE queues in strict issue
  order (the tile scheduler's DMASW sem lanes are queue-locked); gather
  indices are pre-sorted for HBM locality.
"""

import os
import numpy as np
import ml_dtypes

NC_CORES = 8
TILE_P = 128
N_QUEUES = 4
FP8 = ml_dtypes.float8_e4m3


def _round_up(v, m):
    return (v + m - 1) // m * m


def _windows(ntiles):
    """Gather windows (int16-indexable) as tile bounds.  Each window is
    also one AllGather (a Shared tensor has a single writer), so sizes
    taper: big early windows fire mid-layer-1 and their collectives hide
    behind compute; the late windows are small (quick collectives); the
    final 1-tile window keeps the exposed boundary latency tiny."""
    if ntiles <= 2:
        return [0, ntiles]
    b = [0]
    main = ntiles - 1  # reserve the 1-tile final window
    # even-ish thirds up to 18 tiles each: the first window's collective
    # fires early (hides behind layer 1), later ones shrink
    while main - b[-1] > 18:
        b.append(min(b[-1] + 18, main))
    rem = main - b[-1]
    if rem > 14:
        b.append(b[-1] + (rem * 2) // 3)
    b.append(main)
    b.append(ntiles)
    return sorted(set(b))


def _prep_host(x, edge_index):
    """Partition + pad the graph; build per-core stream/gather metadata."""
    x = np.asarray(x, dtype=np.float32)
    edge_index = np.asarray(edge_index)
    N, F = x.shape
    assert N % NC_CORES == 0, (N, NC_CORES)
    npc = N // NC_CORES
    npc_pad = _round_up(npc, TILE_P)
    ntiles = npc_pad // TILE_P
    n_pad = NC_CORES * npc_pad

    loops = np.arange(N, dtype=np.int64)
    # edges WITHOUT self-loops (self term handled on-device)
    row = edge_index[0].astype(np.int64)
    col = edge_index[1].astype(np.int64)
    # degree WITH self-loops (as the reference computes it)
    deg = np.bincount(np.concatenate([col, loops]), minlength=N).astype(np.float64)
    dis = np.where(deg > 0, 1.0 / np.sqrt(deg), 0.0).astype(np.float32)

    src_pad = (row // npc) * npc_pad + (row % npc)
    tgt_core = (col // npc).astype(np.int64)
    tgt_loc = col % npc
    tile_of = tgt_loc // TILE_P
    toff_of = (tgt_loc % TILE_P).astype(np.float32)

    # ---- layer 1: host-expanded stream, grouped by (core, tile) ----
    key1 = tgt_core * ntiles + tile_of
    cnt1 = np.bincount(key1, minlength=NC_CORES * ntiles).reshape(
        NC_CORES, ntiles)
    C1 = _round_up(cnt1.max(axis=0), TILE_P)  # padded counts [ntiles]
    tot1 = int(C1.sum())
    totch1 = tot1 // TILE_P
    nch1 = (C1 // TILE_P).astype(np.int64)
    cs1 = np.zeros(ntiles, dtype=np.int64)
    np.cumsum(nch1[:-1], out=cs1[1:])

    o1 = np.argsort(key1, kind="stable")
    src1_s, toff1_s = src_pad[o1], toff_of[o1]
    g1start = np.zeros(NC_CORES * ntiles + 1, dtype=np.int64)
    np.cumsum(cnt1.reshape(-1), out=g1start[1:])

    # ---- layer 2: dma_gather over W windows (= AllGather pieces) ----
    wb = _windows(ntiles)          # window bounds (tiles)
    W = len(wb) - 1
    asl = wb                       # AG slices coincide with windows
    NS = len(asl) - 1
    win_rows = [(wb[w + 1] - wb[w]) * TILE_P for w in range(W)]
    for r in win_rows:
        assert NC_CORES * r <= 32767, (win_rows,)
    slice_win = np.searchsorted(np.asarray(wb[1:W]),
                                np.asarray(asl[:-1]), side="right")

    src_core = row // npc
    src_loc = row % npc
    src_tile = src_loc // TILE_P
    win = np.searchsorted(np.asarray(wb[1:W]), src_tile, side="right")
    sl = np.searchsorted(np.asarray(asl[1:NS]), src_tile, side="right")
    asl_arr = np.asarray(asl)
    wb_arr = np.asarray(wb)
    rows_s = (asl_arr[sl + 1] - asl_arr[sl]) * TILE_P
    # global row inside the window tensor: slice block offset + core-major
    # position inside the slice
    gidx = (8 * (asl_arr[sl] - wb_arr[win]) * TILE_P
            + src_core * rows_s + (src_loc - asl_arr[sl] * TILE_P))

    key2 = (tgt_core * ntiles + tile_of) * W + win
    cnt2 = np.bincount(key2, minlength=NC_CORES * ntiles * W).reshape(
        NC_CORES, ntiles, W)
    C2 = cnt2.max(axis=0)
    C2 = np.where(C2 > 0, _round_up(C2, TILE_P), 0)  # [ntiles, W]
    tot2 = int(C2.sum())
    cW = (C2 // TILE_P).astype(np.int64)  # chunks per (tile, window)
    nch2 = cW.sum(axis=1)
    cs2 = np.zeros(ntiles, dtype=np.int64)
    np.cumsum(nch2[:-1], out=cs2[1:])

    # secondary sort by source index: the one-hot P absorbs any within-group
    # permutation, and ascending gather addresses improve HBM locality
    o2 = np.lexsort((gidx, key2))
    src2_s, toff2_s = gidx[o2], toff_of[o2]
    g2start = np.zeros(NC_CORES * ntiles * W + 1, dtype=np.int64)
    np.cumsum(cnt2.reshape(-1), out=g2start[1:])

    # padded Dis*x: fp8 copy feeds the edge streams, fp16 the self term
    xs32 = dis[:, None] * x
    xs_pad8 = np.zeros((NC_CORES, npc_pad, F), dtype=FP8)
    xs_pad8[:, :npc] = xs32.reshape(NC_CORES, npc, F).astype(FP8)
    xs_pad16 = np.zeros((NC_CORES, npc_pad, F), dtype=np.float16)
    xs_pad16[:, :npc] = xs32.reshape(NC_CORES, npc, F).astype(np.float16)
    xsl = np.ascontiguousarray(
        xs_pad16.reshape(NC_CORES, ntiles, TILE_P, F).transpose(0, 2, 1, 3))
    xs_flat8 = xs_pad8.reshape(n_pad, F)

    g1 = np.zeros((NC_CORES, 128, totch1, F), dtype=FP8)
    toff1 = np.full((NC_CORES, tot1), -1.0, dtype=np.float32)
    idx2 = np.zeros((NC_CORES, max(tot2, 16)), dtype=np.int16)
    toff2 = np.full((NC_CORES, max(tot2, TILE_P)), -1.0, dtype=np.float32)
    for p in range(NC_CORES):
        off = 0
        for t in range(ntiles):
            g = p * ntiles + t
            a, b = g1start[g], g1start[g + 1]
            n = b - a
            blk = g1[p, :, cs1[t]:cs1[t] + nch1[t], :]
            j = np.arange(n)
            # stream row j -> partition j%128, chunk j//128
            blk[j % 128, j // 128] = xs_flat8[src1_s[a:b]]
            toff1[p, off:off + n] = toff1_s[a:b]
            off += C1[t]
        off = 0
        for t in range(ntiles):
            for h in range(W):
                c = int(C2[t, h])
                if c == 0:
                    continue
                g = (p * ntiles + t) * W + h
                a, b = g2start[g], g2start[g + 1]
                n = b - a
                s = src2_s[a:b]
                assert n <= c and (s >= 0).all() and (s < 32767).all()
                idx2[p, off:off + n] = s.astype(np.int16)
                toff2[p, off:off + n] = toff2_s[a:b]
                off += c

    tot2c = max(tot2, 16)
    idx2_w = np.ascontiguousarray(
        np.tile(idx2.reshape(NC_CORES, tot2c // 16, 16).transpose(0, 2, 1),
                (1, 8, 1)))
    toff1_w = np.ascontiguousarray(
        toff1.reshape(NC_CORES, totch1, TILE_P).transpose(0, 2, 1)).astype(
            np.float16)
    tot2t = max(tot2, TILE_P)
    toff2_w = np.ascontiguousarray(
        toff2.reshape(NC_CORES, tot2t // TILE_P, TILE_P).transpose(0, 2, 1)
    ).astype(np.float16)

    dis_pad = np.zeros((NC_CORES, npc_pad), dtype=np.float32)
    dis_pad[:, :npc] = dis.reshape(NC_CORES, npc)
    dis_tiles = np.ascontiguousarray(
        dis_pad.reshape(NC_CORES, ntiles, TILE_P).transpose(0, 2, 1))
    dis2_tiles = np.ascontiguousarray(dis_tiles * dis_tiles)

    return dict(
        N=N, F=F, npc=npc, npc_pad=npc_pad, ntiles=ntiles, n_pad=n_pad,
        wb=wb, W=W, asl=asl, NS=NS, win_rows=win_rows, slice_win=slice_win,
        nch1=nch1, cs1=cs1, totch1=totch1,
        cW=cW, nch2=nch2, cs2=cs2, tot2=tot2,
        g1=g1.reshape(NC_CORES, 128, totch1 * F),
        xsl=xsl.reshape(NC_CORES, 128, ntiles * F),
        idx2=idx2_w, toff1=toff1_w, toff2=toff2_w,
        dis_tiles=dis_tiles, dis2_tiles=dis2_tiles,
    )


def _build_program(meta, has_b1, has_b2):
    import concourse.bacc as bacc
    import concourse.tile as tile
    from concourse import mybir

    F = meta["F"]
    ntiles = meta["ntiles"]
    npc_pad = meta["npc_pad"]
    wb, W, asl, NS = meta["wb"], meta["W"], meta["asl"], meta["NS"]
    win_rows, slice_win = meta["win_rows"], meta["slice_win"]
    nch1, cs1, totch1 = meta["nch1"], meta["cs1"], meta["totch1"]
    cW, nch2, cs2 = meta["cW"], meta["nch2"], meta["cs2"]
    totw2 = max(meta["tot2"], 16) // 16
    totch2 = max(meta["tot2"], TILE_P) // TILE_P
    nf = F // TILE_P
    f32, f16, i16 = mybir.dt.float32, mybir.dt.float16, mybir.dt.int16
    f8 = mybir.dt.float8e4
    AF = mybir.ActivationFunctionType
    DR = mybir.MatmulPerfMode.DoubleRow

    nc = bacc.Bacc("TRN2", target_bir_lowering=False, debug=False,
                   num_devices=NC_CORES, num_swdge_queues=N_QUEUES)

    g1_d = nc.dram_tensor("g1", [128, totch1 * F], f8, kind="ExternalInput")
    xsl_d = nc.dram_tensor("xsl", [128, ntiles * F], f16, kind="ExternalInput")
    idx_d = nc.dram_tensor("idx", [128, totw2], i16, kind="ExternalInput")
    toff1_d = nc.dram_tensor("toff1", [128, totch1], f16, kind="ExternalInput")
    toff2_d = nc.dram_tensor("toff2", [128, totch2], f16, kind="ExternalInput")
    dis_d = nc.dram_tensor("dis", [128, ntiles], f32, kind="ExternalInput")
    dis2_d = nc.dram_tensor("dis2", [128, ntiles], f32, kind="ExternalInput")
    w1_d = nc.dram_tensor("w1", [F, F], f16, kind="ExternalInput")
    w2_d = nc.dram_tensor("w2", [F, F], f16, kind="ExternalInput")
    id16_d = nc.dram_tensor("id16", [128, 128], f16, kind="ExternalInput")
    iota_d = nc.dram_tensor("iota", [128, 128], f16, kind="ExternalInput")
    if has_b1:
        b1_d = nc.dram_tensor("b1r", [128, F], f32, kind="ExternalInput")
    if has_b2:
        b2_d = nc.dram_tensor("b2r", [128, F], f32, kind="ExternalInput")
    out_d = nc.dram_tensor("out", [npc_pad, F], f16, kind="ExternalOutput")

    eq, add = mybir.AluOpType.is_equal, mybir.AluOpType.add

    with tile.TileContext(nc) as tc:
        with (
            tc.tile_pool(name="const", bufs=1) as cpool,
            tc.tile_pool(name="gbuf", bufs=4) as gpool,
            tc.tile_pool(name="gpre", bufs=3) as prepool,
            tc.tile_pool(name="pbuf", bufs=3) as ppool,
            tc.tile_pool(name="work", bufs=4) as wpool,
            tc.tile_pool(name="h8buf", bufs=3) as hpool,
            tc.tile_pool(name="psA", bufs=3, space="PSUM") as psa,
            tc.tile_pool(name="psB", bufs=2, space="PSUM") as psb,
            tc.tile_pool(name="psC", bufs=3, space="PSUM") as psc,
            tc.tile_pool(name="dram", bufs=1, space="DRAM") as dpool,
        ):
            idx_sb = cpool.tile([128, totw2], i16)
            nc.sync.dma_start(idx_sb[:], idx_d[:, :])
            toff1_sb = cpool.tile([128, totch1], f16)
            nc.sync.dma_start(toff1_sb[:], toff1_d[:, :])
            toff2_sb = cpool.tile([128, totch2], f16)
            nc.sync.dma_start(toff2_sb[:], toff2_d[:, :])
            dis_sb = cpool.tile([128, ntiles], f32)
            nc.sync.dma_start(dis_sb[:], dis_d[:, :])
            dis2_sb = cpool.tile([128, ntiles], f32)
            nc.sync.dma_start(dis2_sb[:], dis2_d[:, :])
            id16_sb = cpool.tile([128, 128], f16)
            nc.sync.dma_start(id16_sb[:], id16_d[:, :])
            iota_sb = cpool.tile([128, 128], f16)
            nc.sync.dma_start(iota_sb[:], iota_d[:, :])
            w1_sb = cpool.tile([128, nf, F], f16)
            w2_sb = cpool.tile([128, nf, F], f16)
            for i in range(nf):
                nc.sync.dma_start(w1_sb[:, i, :], w1_d[128 * i:128 * (i + 1), :])
                nc.sync.dma_start(w2_sb[:, i, :], w2_d[128 * i:128 * (i + 1), :])
            if has_b1:
                b1_sb = cpool.tile([128, F], f32)
                nc.sync.dma_start(b1_sb[:], b1_d[:, :])
            if has_b2:
                b2_sb = cpool.tile([128, F], f32)
                nc.sync.dma_start(b2_sb[:], b2_d[:, :])

            # local shard, fp16: holds Dis*x during layer 1, then Dis*h1
            self_sb = cpool.tile([128, ntiles, F], f16)
            nc.sync.dma_start(
                self_sb[:], xsl_d[:, :].rearrange("p (t f) -> p t f", f=F))

            # per-AG-slice input shards (separate tensors: a slice is only
            # written by its own tiles, so firing its AllGather never
            # serializes against later hs writes), per-WINDOW gather sources
            sl_rows = [(asl[s + 1] - asl[s]) * TILE_P for s in range(NS)]
            hs_shard = [dpool.tile([sl_rows[s], F], f8, name=f"hs_shard{s}")
                        for s in range(NS)]
            hs_win = [dpool.tile([NC_CORES * win_rows[w], F], f8,
                                 addr_space="Shared", name=f"hs_win{w}")
                      for w in range(W)]
            # AG slice -> tile index whose stage_b fires it
            ag_fire = {asl[s + 1] - 1: s for s in range(NS)}
            # tile -> AG slice
            tile_slice = np.searchsorted(
                np.asarray(asl[1:NS]), np.arange(ntiles), side="right")

            def fire_ag(s):
                w = int(slice_win[s])
                off = 8 * (asl[s] - wb[w]) * TILE_P
                nc.gpsimd.collective_compute(
                    "AllGather", mybir.AluOpType.bypass,
                    replica_groups=[list(range(NC_CORES))],
                    ins=[hs_shard[s].opt()],
                    outs=[hs_win[w][off:off + NC_CORES * sl_rows[s], :].opt()])

            PRE = 3  # tiles whose gathers are issued before the main loop
            g_pend = {}
            g_done = {}
            # SWDGE queue must advance in lockstep with issue order: the tile
            # scheduler hands out DMASW sem lanes round-robin per SWDGE
            # instruction, and each sem is locked to one queue — a strict
            # global cycle keeps lane<->queue consistent.
            gq = [0]

            def gather_win(t, w, G):
                cnt = int(cW[t, w])
                if cnt == 0:
                    return
                o_rel = int(cW[t, :w].sum())
                cs = int(cs2[t]) + o_rel
                q = gq[0] % N_QUEUES
                gq[0] += 1
                nc.gpsimd.dma_gather(
                    G[:, o_rel:o_rel + cnt, :], hs_win[w][:, :],
                    idx_sb[:, cs * 8:(cs + cnt) * 8],
                    cnt * 128, cnt * 128, F,
                    single_packet=(cnt * 128 <= 128),
                    queue_num=q)

            def agg_matmuls(aggp, P, G, t, nch):
                """scatter-add: self term (fp16) + fp8 DoubleRow chunk pairs."""
                nc.tensor.matmul(aggp[:], id16_sb[:], self_sb[:, t, :],
                                 start=True, stop=(nch == 0))
                c = 0
                while c < nch:
                    if c + 2 <= nch:
                        nc.tensor.matmul(aggp[:], P[:, c:c + 2, :],
                                         G[:, c:c + 2, :], start=False,
                                         stop=(c + 2 == nch), perf_mode=DR)
                        c += 2
                    else:
                        nc.tensor.matmul(aggp[:], P[:, c, :], G[:, c, :],
                                         start=False, stop=True)
                        c += 1

            for layer in range(2):
                w_sb = w1_sb if layer == 0 else w2_sb
                toff_sb = toff1_sb if layer == 0 else toff2_sb

                def build_p(t):
                    """one-hot matrices for tile t (VectorE), built one tile
                    ahead so the PE never waits on them."""
                    if layer == 0:
                        nch, cs = int(nch1[t]), int(cs1[t])
                    else:
                        nch, cs = int(nch2[t]), int(cs2[t])
                    if not nch:
                        return None
                    P = ppool.tile([128, nch, 128], f8, tag="P")
                    nc.vector.tensor_tensor(
                        P[:],
                        iota_sb[:].unsqueeze(1).broadcast_to([128, nch, 128]),
                        toff_sb[:, cs:cs + nch].unsqueeze(2).broadcast_to(
                            [128, nch, 128]),
                        eq)
                    return P

                def stage_a(t, P):
                    """gather/stream G, scatter-add the incoming messages +
                    self term into PSUM, copy to SBUF (ScalarE)."""
                    if layer == 0:
                        nch, cs = int(nch1[t]), int(cs1[t])
                        G = gpool.tile([128, max(nch, 1), F], f8, tag="G")
                        if nch:
                            nc.sync.dma_start(
                                G[:, 0:nch, :],
                                g1_d[:, cs * F:(cs + nch) * F].rearrange(
                                    "p (c f) -> p c f", f=F))
                    else:
                        nch = int(nch2[t])
                        if t in g_pend:
                            G = g_pend.pop(t)
                            done = g_done.pop(t)
                        else:
                            G = gpool.tile([128, max(nch, 1), F], f8,
                                           tag="G")
                            done = ()
                        # rotate the window issue order per tile: with
                        # W == N_QUEUES a fixed order would pin window w
                        # to queue w permanently (the global queue cycle
                        # advances by W per tile), concentrating the big
                        # window-0 traffic on one queue ring
                        for dw in range(W):
                            w = (dw + t) % W
                            if w not in done:
                                gather_win(t, w, G)
                    # scatter-add (+ self term via identity weights)
                    aggp = psa.tile([128, F], f32, tag="aggp")
                    agg_matmuls(aggp, P, G, t, nch)
                    # PSUM -> SBUF f16 (ScalarE; Dis scaling folded into the
                    # final activation instead)
                    aggc = wpool.tile([128, F], f16, tag="aggc")
                    nc.scalar.activation(aggc[:], aggp[:], AF.Copy)
                    return aggc

                def stage_t(t, aggc):
                    """TensorE transpose of the aggregate + copy out of PSUM."""
                    pT = psb.tile([128, F], f16, tag="pT")
                    for i in range(nf):
                        nc.tensor.transpose(pT[:, 128 * i:128 * (i + 1)],
                                            aggc[:, 128 * i:128 * (i + 1)],
                                            id16_sb[:])
                    aggT = wpool.tile([128, nf, 128], f16, tag="aggT")
                    nc.vector.tensor_copy(
                        aggT[:].rearrange("p a b -> p (a b)"), pT[:])
                    return aggT

                def stage_b(t, aggT):
                    """dense weight matmul + scaled activation + writeback."""
                    zp = psc.tile([128, F], f32, tag="zp")
                    for i in range(nf):
                        nc.tensor.matmul(zp[:], aggT[:, i, :], w_sb[:, i, :],
                                         start=(i == 0), stop=(i == nf - 1))
                    r0, r1 = TILE_P * t, TILE_P * (t + 1)
                    if layer == 0:
                        zin = zp[:]
                        if has_b1:
                            zb = wpool.tile([128, F], f32, tag="zb")
                            nc.vector.tensor_tensor(zb[:], zp[:], b1_sb[:], add)
                            zin = zb[:]
                        # self_sb[t] := dis^2 * relu(z) == dis * relu(dis * z)
                        nc.scalar.activation(self_sb[:, t, :], zin, AF.Relu,
                                             scale=dis2_sb[:, t:t + 1])
                        # fp8 copy of the same activation for the AllGather /
                        # layer-2 gather stream
                        h8 = hpool.tile([128, F], f8, tag="h8")
                        nc.scalar.activation(h8[:], zin, AF.Relu,
                                             scale=dis2_sb[:, t:t + 1])
                        s = int(tile_slice[t])
                        b0 = (t - asl[s]) * TILE_P
                        # hs writes ride the ScalarE HWDGE queue: off the
                        # SWDGE lanes (whose sem rotation the gathers own)
                        # and off the SP ring (so the g1 stream never waits)
                        nc.scalar.dma_start(hs_shard[s][b0:b0 + TILE_P, :],
                                            h8[:])
                        if t in ag_fire:
                            k = ag_fire[t]
                            fire_ag(k)
                            # prefetch the first tiles' gathers once their
                            # window's AllGather slices have all fired: the
                            # non-final windows at the second-to-last fire
                            # (their data landed long ago — no engine-
                            # blocking waits before the final AG fires), the
                            # last window right after the final fire.  A
                            # dedicated pool keeps these allocations out of
                            # the main G ring.
                            if k == NS - 2 and NS >= 2 and \
                                    asl[k + 1] >= ntiles - 1:
                                for tt in range(min(PRE, ntiles)):
                                    nch_t = int(nch2[tt])
                                    G = prepool.tile([128, max(nch_t, 1), F],
                                                     f8, tag="Gpre")
                                    g_pend[tt] = G
                                    g_done[tt] = set()
                                    # only windows whose AllGather landed
                                    # long ago: a not-yet-landed window
                                    # would block the gpsimd engine before
                                    # the final AllGather fires
                                    for w in range(max(W - 3, 0)):
                                        gather_win(tt, w, G)
                                        g_done[tt].add(w)
                            elif k == NS - 1:
                                if not g_pend:
                                    for tt in range(min(PRE, ntiles)):
                                        nch_t = int(nch2[tt])
                                        G = prepool.tile(
                                            [128, max(nch_t, 1), F],
                                            f8, tag="Gpre")
                                        g_pend[tt] = G
                                        g_done[tt] = set()
                                for tt in sorted(g_pend):
                                    for w in range(W):
                                        if w not in g_done[tt]:
                                            gather_win(tt, w, g_pend[tt])
                                            g_done[tt].add(w)
                    else:
                        o_t = wpool.tile([128, F], f16, tag="ot")
                        zin = zp[:]
                        if has_b2:
                            zb = wpool.tile([128, F], f32, tag="zb")
                            nc.vector.tensor_tensor(zb[:], zp[:], b2_sb[:], add)
                            zin = zb[:]
                        # out := dis * z  (SP ring is idle in layer 2)
                        nc.scalar.activation(o_t[:], zin, AF.Copy,
                                             scale=dis_sb[:, t:t + 1])
                        nc.sync.dma_start(out_d[r0:r1, :], o_t[:])

                # 2-deep software pipeline: PE order is aggp(t), T(t-1),
                # zp(t-2) so the TensorE stream never stalls on the
                # cross-engine transpose round-trip; P built one tile ahead
                p_next = build_p(0)
                aggc_q, aggt_q = {}, {}
                for t in range(ntiles + 2):
                    if t < ntiles:
                        P_cur = p_next
                        p_next = build_p(t + 1) if t + 1 < ntiles else None
                        aggc_q[t] = stage_a(t, P_cur)
                    if 1 <= t <= ntiles:
                        aggt_q[t - 1] = stage_t(t - 1, aggc_q.pop(t - 1))
                    if t >= 2:
                        stage_b(t - 2, aggt_q.pop(t - 2))

    nc.compile()
    return nc


def kernel(x, edge_index, W1, b1, W2, b2):
    x = np.asarray(x, dtype=np.float32)
    W1 = np.asarray(W1, dtype=np.float32)
    W2 = np.asarray(W2, dtype=np.float32)
    b1 = np.asarray(b1, dtype=np.float32)
    b2 = np.asarray(b2, dtype=np.float32)
    meta = _prep_host(x, edge_index)

    has_b1 = bool(np.any(b1))
    has_b2 = bool(np.any(b2))
    nc = _build_program(meta, has_b1, has_b2)

    in_maps = []
    for p in range(NC_CORES):
        m = {
            "g1": meta["g1"][p],
            "xsl": meta["xsl"][p],
            "idx": meta["idx2"][p],
            "toff1": meta["toff1"][p],
            "toff2": meta["toff2"][p],
            "dis": meta["dis_tiles"][p],
            "dis2": meta["dis2_tiles"][p],
            "w1": W1.astype(np.float16), "w2": W2.astype(np.float16),
            "id16": np.eye(128, dtype=np.float16),
            "iota": np.tile(np.arange(128, dtype=np.float16), (128, 1)),
        }
        if has_b1:
            m["b1r"] = np.tile(b1, (128, 1)).astype(np.float32)
        if has_b2:
            m["b2r"] = np.tile(b2, (128, 1)).astype(np.float32)
        in_maps.append(m)

    if os.environ.get("GNN_SIM", "0") == "1":
        from concourse.bass_interp import MultiCoreSim
        sim = MultiCoreSim(nc, num_cores=NC_CORES, trace=False)
        cores = list(sim.cores.values())
        for p, core in enumerate(cores):
            for k, v in in_maps[p].items():
                core.tensor(k)[:] = v
        sim.simulate(check_with_hw=False)
        shards = [cores[p].tensor("out").copy() for p in range(NC_CORES)]
    else:
        from concourse import bass_utils
        trace = os.environ.get("GNN_TRACE", "0") == "1"
        res = bass_utils.run_bass_kernel_spmd(
            nc, in_maps, core_ids=list(range(NC_CORES)), trace=trace)
        if trace and res.exec_time_ns is not None:
            print(f"HW exec time: {res.exec_time_ns} ns")
        kernel.last_results = res
        shards = [res.results[p]["out"] for p in range(NC_CORES)]

    npc = meta["npc"]
    out = np.concatenate([s[:npc] for s in shards], axis=0)
    return out.astype(np.float32)


# revision 21
# speedup vs baseline: 1.0421x; 1.0022x over previous
"""Trainium2 Bass kernel: 2-layer GCN (GCNConv -> ReLU -> GCNConv).

Math:  S = D^-1/2 (A + I) D^-1/2  (A from edge_index, self-loops appended)
       out = S @ relu(S @ x @ W1 + b1) @ W2 + b2
Using linearity, aggregate-then-matmul per layer with u = Dis*x:
       agg1 = A' @ u + u            (A' = adjacency without self-loops)
       h~   = Dis^2 * relu(agg1 @ W1 (+ b1))   (= Dis * h1, stored fp16)
       agg2 = A' @ h~ + h~
       out  = Dis * (agg2 @ W2) (+ b2)
The Dis row-scaling commutes with the dense matmul, so it is folded into a
single ScalarE activation per tile (scale = dis^2 resp. dis, func=relu/copy).

Distribution: nodes sharded over 8 NeuronCores.  Per layer, each core
scatter-adds incoming-edge source rows per 128-target tile with TensorE
matmuls against one-hot matrices (built on VectorE via is_equal vs iota, one
tile ahead of use), transposes via TensorE and applies the dense weight
matmul in fp16, all on a 2-deep software pipeline (PE order: aggregate(t),
transpose(t-1), dense(t-2)) so the TensorE stream never stalls.

Precision: the gathered/streamed neighbor rows, the one-hot matrices and the
AllGather payload are fp8e4 (self term, weights and dense path stay fp16;
measured HW rel-err 1.8e-2 vs the 2e-2 gate).  The scatter-add runs two fp8
chunks per PE pass with MatmulPerfMode.DoubleRow (2x fp8 throughput), so
both HBM traffic and aggregation PE time halve vs fp16.
  Layer 1: gather indices are static and the source data (fp8 Dis*x) is a
  kernel input, so the HOST pre-expands the gathered stream into edge order;
  the device streams it sequentially over HWDGE at full HBM bandwidth.
  Layer 2: fp8 activations are AllGathered into TWO window tensors (each
  int16-indexable: 8*window_rows <= 32767) so each tile needs only two
  dma_gather calls, but the collective itself is cut into ~8 small SLICES
  (separate input shard tensors, sliced output APs) fired as soon as each
  slice's tiles are computed — the CC stream pipelines behind layer-1
  compute and only a tiny final slice is exposed at the layer boundary.
  Gather descriptor generation cycles the 4 SWDGE queues in strict issue
  order (the tile scheduler's DMASW sem lanes are queue-locked); gather
  indices are pre-sorted for HBM locality.
"""

import os
import numpy as np
import ml_dtypes

NC_CORES = 8
TILE_P = 128
N_QUEUES = 4
FP8 = ml_dtypes.float8_e4m3


def _round_up(v, m):
    return (v + m - 1) // m * m


def _windows(ntiles):
    """Gather windows (int16-indexable) as tile bounds.  Each window is
    also one AllGather (a Shared tensor has a single writer), so sizes
    taper: big early windows fire mid-layer-1 and their collectives hide
    behind compute; the late windows are small (quick collectives); the
    final 1-tile window keeps the exposed boundary latency tiny."""
    if ntiles <= 2:
        return [0, ntiles]
    b = [0]
    main = ntiles - 1  # reserve the 1-tile final window
    # even-ish thirds up to 18 tiles each: the first window's collective
    # fires early (hides behind layer 1), later ones shrink
    while main - b[-1] > 18:
        b.append(min(b[-1] + 18, main))
    rem = main - b[-1]
    if rem > 14:
        b.append(b[-1] + (rem * 2) // 3)
    b.append(main)
    b.append(ntiles)
    return sorted(set(b))


def _prep_host(x, edge_index):
    """Partition + pad the graph; build per-core stream/gather metadata."""
    x = np.asarray(x, dtype=np.float32)
    edge_index = np.asarray(edge_index)
    N, F = x.shape
    assert N % NC_CORES == 0, (N, NC_CORES)
    npc = N // NC_CORES
    npc_pad = _round_up(npc, TILE_P)
    ntiles = npc_pad // TILE_P
    n_pad = NC_CORES * npc_pad

    loops = np.arange(N, dtype=np.int64)
    # edges WITHOUT self-loops (self term handled on-device)
    row = edge_index[0].astype(np.int64)
    col = edge_index[1].astype(np.int64)
    # degree WITH self-loops (as the reference computes it)
    deg = np.bincount(np.concatenate([col, loops]), minlength=N).astype(np.float64)
    dis = np.where(deg > 0, 1.0 / np.sqrt(deg), 0.0).astype(np.float32)

    src_pad = (row // npc) * npc_pad + (row % npc)
    tgt_core = (col // npc).astype(np.int64)
    tgt_loc = col % npc
    tile_of = tgt_loc // TILE_P
    toff_of = (tgt_loc % TILE_P).astype(np.float32)

    # ---- layer 1: host-expanded stream, grouped by (core, tile) ----
    key1 = tgt_core * ntiles + tile_of
    cnt1 = np.bincount(key1, minlength=NC_CORES * ntiles).reshape(
        NC_CORES, ntiles)
    C1 = _round_up(cnt1.max(axis=0), TILE_P)  # padded counts [ntiles]
    tot1 = int(C1.sum())
    totch1 = tot1 // TILE_P
    nch1 = (C1 // TILE_P).astype(np.int64)
    cs1 = np.zeros(ntiles, dtype=np.int64)
    np.cumsum(nch1[:-1], out=cs1[1:])

    o1 = np.argsort(key1, kind="stable")
    src1_s, toff1_s = src_pad[o1], toff_of[o1]
    g1start = np.zeros(NC_CORES * ntiles + 1, dtype=np.int64)
    np.cumsum(cnt1.reshape(-1), out=g1start[1:])

    # ---- layer 2: dma_gather over W windows (= AllGather pieces) ----
    wb = _windows(ntiles)          # window bounds (tiles)
    W = len(wb) - 1
    asl = wb                       # AG slices coincide with windows
    NS = len(asl) - 1
    win_rows = [(wb[w + 1] - wb[w]) * TILE_P for w in range(W)]
    for r in win_rows:
        assert NC_CORES * r <= 32767, (win_rows,)
    slice_win = np.searchsorted(np.asarray(wb[1:W]),
                                np.asarray(asl[:-1]), side="right")

    src_core = row // npc
    src_loc = row % npc
    src_tile = src_loc // TILE_P
    win = np.searchsorted(np.asarray(wb[1:W]), src_tile, side="right")
    sl = np.searchsorted(np.asarray(asl[1:NS]), src_tile, side="right")
    asl_arr = np.asarray(asl)
    wb_arr = np.asarray(wb)
    rows_s = (asl_arr[sl + 1] - asl_arr[sl]) * TILE_P
    # global row inside the window tensor: slice block offset + core-major
    # position inside the slice
    gidx = (8 * (asl_arr[sl] - wb_arr[win]) * TILE_P
            + src_core * rows_s + (src_loc - asl_arr[sl] * TILE_P))

    key2 = (tgt_core * ntiles + tile_of) * W + win
    cnt2 = np.bincount(key2, minlength=NC_CORES * ntiles * W).reshape(
        NC_CORES, ntiles, W)
    C2 = cnt2.max(axis=0)
    C2 = np.where(C2 > 0, _round_up(C2, TILE_P), 0)  # [ntiles, W]
    tot2 = int(C2.sum())
    cW = (C2 // TILE_P).astype(np.int64)  # chunks per (tile, window)
    nch2 = cW.sum(axis=1)
    cs2 = np.zeros(ntiles, dtype=np.int64)
    np.cumsum(nch2[:-1], out=cs2[1:])

    # secondary sort by source index: the one-hot P absorbs any within-group
    # permutation, and ascending gather addresses improve HBM locality
    o2 = np.lexsort((gidx, key2))
    src2_s, toff2_s = gidx[o2], toff_of[o2]
    g2start = np.zeros(NC_CORES * ntiles * W + 1, dtype=np.int64)
    np.cumsum(cnt2.reshape(-1), out=g2start[1:])

    # pair adjacent tiles: one G buffer and one gather call per (pair,
    # window) halves the SWDGE call count (the gpsimd engine's serial
    # per-call prep is the layer-2 pacing cost).  G chunk layout per pair
    # is window-major: [w0: t0|t1, w1: t0|t1, ...]; P stays per-tile, the
    # matmul loop maps P-chunk ranges onto G-chunk ranges explicitly.
    npairs = (ntiles + 1) // 2
    pair_tiles = [[t for t in (2 * q, 2 * q + 1) if t < ntiles]
                  for q in range(npairs)]
    gwb = np.zeros((npairs, W + 1), dtype=np.int64)   # pair-local chunk base
    gloc = np.zeros((ntiles, W), dtype=np.int64)      # tile's base inside it
    nchp = np.zeros(npairs, dtype=np.int64)
    for q in range(npairs):
        o = 0
        for w in range(W):
            gwb[q, w] = o
            for t in pair_tiles[q]:
                gloc[t, w] = o
                o += cW[t, w]
        gwb[q, W] = o
        nchp[q] = o
    ib = np.zeros((npairs, W), dtype=np.int64)        # idx-stream chunk base
    o = 0
    for q in range(npairs):
        for w in range(W):
            ib[q, w] = o
            for t in pair_tiles[q]:
                o += cW[t, w]

    # padded Dis*x: fp8 copy feeds the edge streams, fp16 the self term
    xs32 = dis[:, None] * x
    xs_pad8 = np.zeros((NC_CORES, npc_pad, F), dtype=FP8)
    xs_pad8[:, :npc] = xs32.reshape(NC_CORES, npc, F).astype(FP8)
    xs_pad16 = np.zeros((NC_CORES, npc_pad, F), dtype=np.float16)
    xs_pad16[:, :npc] = xs32.reshape(NC_CORES, npc, F).astype(np.float16)
    xsl = np.ascontiguousarray(
        xs_pad16.reshape(NC_CORES, ntiles, TILE_P, F).transpose(0, 2, 1, 3))
    xs_flat8 = xs_pad8.reshape(n_pad, F)

    g1 = np.zeros((NC_CORES, 128, totch1, F), dtype=FP8)
    toff1 = np.full((NC_CORES, tot1), -1.0, dtype=np.float32)
    idx2 = np.zeros((NC_CORES, max(tot2, 16)), dtype=np.int16)
    toff2 = np.full((NC_CORES, max(tot2, TILE_P)), -1.0, dtype=np.float32)
    for p in range(NC_CORES):
        off = 0
        for t in range(ntiles):
            g = p * ntiles + t
            a, b = g1start[g], g1start[g + 1]
            n = b - a
            blk = g1[p, :, cs1[t]:cs1[t] + nch1[t], :]
            j = np.arange(n)
            # stream row j -> partition j%128, chunk j//128
            blk[j % 128, j // 128] = xs_flat8[src1_s[a:b]]
            toff1[p, off:off + n] = toff1_s[a:b]
            off += C1[t]
        # toff stream: per-tile window-major (matches the P build slices)
        off = 0
        for t in range(ntiles):
            for h in range(W):
                c = int(C2[t, h])
                if c == 0:
                    continue
                g = (p * ntiles + t) * W + h
                a, b = g2start[g], g2start[g + 1]
                toff2[p, off:off + (b - a)] = toff2_s[a:b]
                off += c
        # idx stream: per-pair window-major (matches the G chunk layout)
        off = 0
        for q in range(npairs):
            for h in range(W):
                for t in pair_tiles[q]:
                    c = int(C2[t, h])
                    if c == 0:
                        continue
                    g = (p * ntiles + t) * W + h
                    a, b = g2start[g], g2start[g + 1]
                    n = b - a
                    s = src2_s[a:b]
                    assert n <= c and (s >= 0).all() and (s < 32767).all()
                    idx2[p, off:off + n] = s.astype(np.int16)
                    off += c

    tot2c = max(tot2, 16)
    idx2_w = np.ascontiguousarray(
        np.tile(idx2.reshape(NC_CORES, tot2c // 16, 16).transpose(0, 2, 1),
                (1, 8, 1)))
    toff1_w = np.ascontiguousarray(
        toff1.reshape(NC_CORES, totch1, TILE_P).transpose(0, 2, 1)).astype(
            np.float16)
    tot2t = max(tot2, TILE_P)
    toff2_w = np.ascontiguousarray(
        toff2.reshape(NC_CORES, tot2t // TILE_P, TILE_P).transpose(0, 2, 1)
    ).astype(np.float16)

    dis_pad = np.zeros((NC_CORES, npc_pad), dtype=np.float32)
    dis_pad[:, :npc] = dis.reshape(NC_CORES, npc)
    dis_tiles = np.ascontiguousarray(
        dis_pad.reshape(NC_CORES, ntiles, TILE_P).transpose(0, 2, 1))
    dis2_tiles = np.ascontiguousarray(dis_tiles * dis_tiles)

    return dict(
        N=N, F=F, npc=npc, npc_pad=npc_pad, ntiles=ntiles, n_pad=n_pad,
        wb=wb, W=W, asl=asl, NS=NS, win_rows=win_rows, slice_win=slice_win,
        nch1=nch1, cs1=cs1, totch1=totch1,
        cW=cW, nch2=nch2, cs2=cs2, tot2=tot2,
        npairs=npairs, pair_tiles=pair_tiles, gwb=gwb, gloc=gloc,
        nchp=nchp, ib=ib,
        g1=g1.reshape(NC_CORES, 128, totch1 * F),
        xsl=xsl.reshape(NC_CORES, 128, ntiles * F),
        idx2=idx2_w, toff1=toff1_w, toff2=toff2_w,
        dis_tiles=dis_tiles, dis2_tiles=dis2_tiles,
    )


def _build_program(meta, has_b1, has_b2):
    import concourse.bacc as bacc
    import concourse.tile as tile
    from concourse import mybir

    F = meta["F"]
    ntiles = meta["ntiles"]
    npc_pad = meta["npc_pad"]
    wb, W, asl, NS = meta["wb"], meta["W"], meta["asl"], meta["NS"]
    win_rows, slice_win = meta["win_rows"], meta["slice_win"]
    nch1, cs1, totch1 = meta["nch1"], meta["cs1"], meta["totch1"]
    cW, nch2, cs2 = meta["cW"], meta["nch2"], meta["cs2"]
    npairs, pair_tiles = meta["npairs"], meta["pair_tiles"]
    gwb, gloc, nchp, ib = meta["gwb"], meta["gloc"], meta["nchp"], meta["ib"]
    totw2 = max(meta["tot2"], 16) // 16
    totch2 = max(meta["tot2"], TILE_P) // TILE_P
    nf = F // TILE_P
    f32, f16, i16 = mybir.dt.float32, mybir.dt.float16, mybir.dt.int16
    f8 = mybir.dt.float8e4
    AF = mybir.ActivationFunctionType
    DR = mybir.MatmulPerfMode.DoubleRow

    nc = bacc.Bacc("TRN2", target_bir_lowering=False, debug=False,
                   num_devices=NC_CORES, num_swdge_queues=N_QUEUES)

    g1_d = nc.dram_tensor("g1", [128, totch1 * F], f8, kind="ExternalInput")
    xsl_d = nc.dram_tensor("xsl", [128, ntiles * F], f16, kind="ExternalInput")
    idx_d = nc.dram_tensor("idx", [128, totw2], i16, kind="ExternalInput")
    toff1_d = nc.dram_tensor("toff1", [128, totch1], f16, kind="ExternalInput")
    toff2_d = nc.dram_tensor("toff2", [128, totch2], f16, kind="ExternalInput")
    dis_d = nc.dram_tensor("dis", [128, ntiles], f32, kind="ExternalInput")
    dis2_d = nc.dram_tensor("dis2", [128, ntiles], f32, kind="ExternalInput")
    w1_d = nc.dram_tensor("w1", [F, F], f16, kind="ExternalInput")
    w2_d = nc.dram_tensor("w2", [F, F], f16, kind="ExternalInput")
    id16_d = nc.dram_tensor("id16", [128, 128], f16, kind="ExternalInput")
    iota_d = nc.dram_tensor("iota", [128, 128], f16, kind="ExternalInput")
    if has_b1:
        b1_d = nc.dram_tensor("b1r", [128, F], f32, kind="ExternalInput")
    if has_b2:
        b2_d = nc.dram_tensor("b2r", [128, F], f32, kind="ExternalInput")
    out_d = nc.dram_tensor("out", [npc_pad, F], f16, kind="ExternalOutput")

    eq, add = mybir.AluOpType.is_equal, mybir.AluOpType.add

    with tile.TileContext(nc) as tc:
        with (
            tc.tile_pool(name="const", bufs=1) as cpool,
            tc.tile_pool(name="gbuf", bufs=3) as gpool,
            tc.tile_pool(name="gpre", bufs=2) as prepool,
            tc.tile_pool(name="pbuf", bufs=3) as ppool,
            tc.tile_pool(name="work", bufs=4) as wpool,
            tc.tile_pool(name="h8buf", bufs=3) as hpool,
            tc.tile_pool(name="psA", bufs=3, space="PSUM") as psa,
            tc.tile_pool(name="psB", bufs=2, space="PSUM") as psb,
            tc.tile_pool(name="psC", bufs=3, space="PSUM") as psc,
            tc.tile_pool(name="dram", bufs=1, space="DRAM") as dpool,
        ):
            idx_sb = cpool.tile([128, totw2], i16)
            nc.sync.dma_start(idx_sb[:], idx_d[:, :])
            toff1_sb = cpool.tile([128, totch1], f16)
            nc.sync.dma_start(toff1_sb[:], toff1_d[:, :])
            toff2_sb = cpool.tile([128, totch2], f16)
            nc.sync.dma_start(toff2_sb[:], toff2_d[:, :])
            dis_sb = cpool.tile([128, ntiles], f32)
            nc.sync.dma_start(dis_sb[:], dis_d[:, :])
            dis2_sb = cpool.tile([128, ntiles], f32)
            nc.sync.dma_start(dis2_sb[:], dis2_d[:, :])
            id16_sb = cpool.tile([128, 128], f16)
            nc.sync.dma_start(id16_sb[:], id16_d[:, :])
            iota_sb = cpool.tile([128, 128], f16)
            nc.sync.dma_start(iota_sb[:], iota_d[:, :])
            w1_sb = cpool.tile([128, nf, F], f16)
            w2_sb = cpool.tile([128, nf, F], f16)
            for i in range(nf):
                nc.sync.dma_start(w1_sb[:, i, :], w1_d[128 * i:128 * (i + 1), :])
                nc.sync.dma_start(w2_sb[:, i, :], w2_d[128 * i:128 * (i + 1), :])
            if has_b1:
                b1_sb = cpool.tile([128, F], f32)
                nc.sync.dma_start(b1_sb[:], b1_d[:, :])
            if has_b2:
                b2_sb = cpool.tile([128, F], f32)
                nc.sync.dma_start(b2_sb[:], b2_d[:, :])

            # local shard, fp16: holds Dis*x during layer 1, then Dis*h1
            self_sb = cpool.tile([128, ntiles, F], f16)
            nc.sync.dma_start(
                self_sb[:], xsl_d[:, :].rearrange("p (t f) -> p t f", f=F))

            # per-AG-slice input shards (separate tensors: a slice is only
            # written by its own tiles, so firing its AllGather never
            # serializes against later hs writes), per-WINDOW gather sources
            sl_rows = [(asl[s + 1] - asl[s]) * TILE_P for s in range(NS)]
            hs_shard = [dpool.tile([sl_rows[s], F], f8, name=f"hs_shard{s}")
                        for s in range(NS)]
            hs_win = [dpool.tile([NC_CORES * win_rows[w], F], f8,
                                 addr_space="Shared", name=f"hs_win{w}")
                      for w in range(W)]
            # AG slice -> tile index whose stage_b fires it
            ag_fire = {asl[s + 1] - 1: s for s in range(NS)}
            # tile -> AG slice
            tile_slice = np.searchsorted(
                np.asarray(asl[1:NS]), np.arange(ntiles), side="right")

            def fire_ag(s):
                w = int(slice_win[s])
                off = 8 * (asl[s] - wb[w]) * TILE_P
                nc.gpsimd.collective_compute(
                    "AllGather", mybir.AluOpType.bypass,
                    replica_groups=[list(range(NC_CORES))],
                    ins=[hs_shard[s].opt()],
                    outs=[hs_win[w][off:off + NC_CORES * sl_rows[s], :].opt()])

            PREP = 2  # tile PAIRS whose gathers precede the main loop
            g_pend = {}
            # SWDGE queue must advance in lockstep with issue order: the tile
            # scheduler hands out DMASW sem lanes round-robin per SWDGE
            # instruction, and each sem is locked to one queue — a strict
            # global cycle keeps lane<->queue consistent.
            gq = [0]

            def gather_pair(q_pair, w, G):
                """one call fetches BOTH paired tiles' window-w chunks."""
                cnt = sum(int(cW[t, w]) for t in pair_tiles[q_pair])
                if cnt == 0:
                    return
                gw = int(gwb[q_pair, w])
                cs = int(ib[q_pair, w])
                q = gq[0] % N_QUEUES
                gq[0] += 1
                nc.gpsimd.dma_gather(
                    G[:, gw:gw + cnt, :], hs_win[w][:, :],
                    idx_sb[:, cs * 8:(cs + cnt) * 8],
                    cnt * 128, cnt * 128, F,
                    single_packet=(cnt * 128 <= 128),
                    queue_num=q)

            def agg_matmuls(aggp, P, G, t, ranges):
                """scatter-add: self term (fp16) + fp8 DoubleRow chunk pairs.
                ranges: (P-chunk base, G-chunk base, count) per window."""
                tot = sum(r[2] for r in ranges)
                nc.tensor.matmul(aggp[:], id16_sb[:], self_sb[:, t, :],
                                 start=True, stop=(tot == 0))
                done = 0
                for pb, gb, cnt in ranges:
                    c = 0
                    while c < cnt:
                        step = 2 if c + 2 <= cnt else 1
                        stop = (done + c + step == tot)
                        if step == 2:
                            nc.tensor.matmul(
                                aggp[:], P[:, pb + c:pb + c + 2, :],
                                G[:, gb + c:gb + c + 2, :], start=False,
                                stop=stop, perf_mode=DR)
                        else:
                            nc.tensor.matmul(
                                aggp[:], P[:, pb + c, :], G[:, gb + c, :],
                                start=False, stop=stop)
                        c += step
                    done += cnt

            for layer in range(2):
                w_sb = w1_sb if layer == 0 else w2_sb
                toff_sb = toff1_sb if layer == 0 else toff2_sb

                def build_p(t):
                    """one-hot matrices for tile t (VectorE), built one tile
                    ahead so the PE never waits on them."""
                    if layer == 0:
                        nch, cs = int(nch1[t]), int(cs1[t])
                    else:
                        nch, cs = int(nch2[t]), int(cs2[t])
                    if not nch:
                        return None
                    P = ppool.tile([128, nch, 128], f8, tag="P")
                    nc.vector.tensor_tensor(
                        P[:],
                        iota_sb[:].unsqueeze(1).broadcast_to([128, nch, 128]),
                        toff_sb[:, cs:cs + nch].unsqueeze(2).broadcast_to(
                            [128, nch, 128]),
                        eq)
                    return P

                def stage_a(t, P):
                    """gather/stream G, scatter-add the incoming messages +
                    self term into PSUM, copy to SBUF (ScalarE)."""
                    if layer == 0:
                        nch, cs = int(nch1[t]), int(cs1[t])
                        G = gpool.tile([128, max(nch, 1), F], f8, tag="G")
                        if nch:
                            nc.sync.dma_start(
                                G[:, 0:nch, :],
                                g1_d[:, cs * F:(cs + nch) * F].rearrange(
                                    "p (c f) -> p c f", f=F))
                        ranges = [(0, 0, nch)] if nch else []
                    else:
                        qp = t // 2
                        if qp in g_pend:
                            G, done = g_pend[qp]
                        else:
                            G = gpool.tile([128, max(int(nchp[qp]), 1), F],
                                           f8, tag="G")
                            done = set()
                            g_pend[qp] = (G, done)
                        # rotate the window issue order per tile: a fixed
                        # order would pin the big window-0 traffic to one
                        # queue ring (the global queue cycle advances by a
                        # fixed amount per tile)
                        for dw in range(W):
                            w = (dw + t) % W
                            if w not in done:
                                gather_pair(qp, w, G)
                                done.add(w)
                        if t == pair_tiles[qp][-1]:
                            g_pend.pop(qp)
                        ranges = []
                        pb = 0
                        for w in range(W):
                            cnt = int(cW[t, w])
                            if cnt:
                                ranges.append((pb, int(gloc[t, w]), cnt))
                            pb += cnt
                    # scatter-add (+ self term via identity weights)
                    aggp = psa.tile([128, F], f32, tag="aggp")
                    agg_matmuls(aggp, P, G, t, ranges)
                    # PSUM -> SBUF f16 (ScalarE; Dis scaling folded into the
                    # final activation instead)
                    aggc = wpool.tile([128, F], f16, tag="aggc")
                    nc.scalar.activation(aggc[:], aggp[:], AF.Copy)
                    return aggc

                def stage_t(t, aggc):
                    """TensorE transpose of the aggregate + copy out of PSUM."""
                    pT = psb.tile([128, F], f16, tag="pT")
                    for i in range(nf):
                        nc.tensor.transpose(pT[:, 128 * i:128 * (i + 1)],
                                            aggc[:, 128 * i:128 * (i + 1)],
                                            id16_sb[:])
                    aggT = wpool.tile([128, nf, 128], f16, tag="aggT")
                    nc.vector.tensor_copy(
                        aggT[:].rearrange("p a b -> p (a b)"), pT[:])
                    return aggT

                def stage_b(t, aggT):
                    """dense weight matmul + scaled activation + writeback."""
                    zp = psc.tile([128, F], f32, tag="zp")
                    for i in range(nf):
                        nc.tensor.matmul(zp[:], aggT[:, i, :], w_sb[:, i, :],
                                         start=(i == 0), stop=(i == nf - 1))
                    r0, r1 = TILE_P * t, TILE_P * (t + 1)
                    if layer == 0:
                        zin = zp[:]
                        if has_b1:
                            zb = wpool.tile([128, F], f32, tag="zb")
                            nc.vector.tensor_tensor(zb[:], zp[:], b1_sb[:], add)
                            zin = zb[:]
                        # self_sb[t] := dis^2 * relu(z) == dis * relu(dis * z)
                        nc.scalar.activation(self_sb[:, t, :], zin, AF.Relu,
                                             scale=dis2_sb[:, t:t + 1])
                        # fp8 copy of the same activation for the AllGather /
                        # layer-2 gather stream
                        h8 = hpool.tile([128, F], f8, tag="h8")
                        nc.scalar.activation(h8[:], zin, AF.Relu,
                                             scale=dis2_sb[:, t:t + 1])
                        s = int(tile_slice[t])
                        b0 = (t - asl[s]) * TILE_P
                        # hs writes ride the ScalarE HWDGE queue: off the
                        # SWDGE lanes (whose sem rotation the gathers own)
                        # and off the SP ring (so the g1 stream never waits)
                        nc.scalar.dma_start(hs_shard[s][b0:b0 + TILE_P, :],
                                            h8[:])
                        if t in ag_fire:
                            k = ag_fire[t]
                            fire_ag(k)
                            # prefetch the first tiles' gathers once their
                            # window's AllGather slices have all fired: the
                            # non-final windows at the second-to-last fire
                            # (their data landed long ago — no engine-
                            # blocking waits before the final AG fires), the
                            # last window right after the final fire.  A
                            # dedicated pool keeps these allocations out of
                            # the main G ring.
                            if k == NS - 2 and NS >= 2 and \
                                    asl[k + 1] >= ntiles - 1:
                                for qq in range(min(PREP, npairs)):
                                    G = prepool.tile(
                                        [128, max(int(nchp[qq]), 1), F],
                                        f8, tag="Gpre")
                                    done = set()
                                    g_pend[qq] = (G, done)
                                    # only windows whose AllGather landed
                                    # long ago: a not-yet-landed window
                                    # would block the gpsimd engine before
                                    # the final AllGather fires
                                    for w in range(max(W - 3, 0)):
                                        gather_pair(qq, w, G)
                                        done.add(w)
                            elif k == NS - 1:
                                if not g_pend:
                                    for qq in range(min(PREP, npairs)):
                                        G = prepool.tile(
                                            [128, max(int(nchp[qq]), 1), F],
                                            f8, tag="Gpre")
                                        g_pend[qq] = (G, set())
                                for qq in sorted(g_pend):
                                    G, done = g_pend[qq]
                                    for w in range(W):
                                        if w not in done:
                                            gather_pair(qq, w, G)
                                            done.add(w)
                    else:
                        o_t = wpool.tile([128, F], f16, tag="ot")
                        zin = zp[:]
                        if has_b2:
                            zb = wpool.tile([128, F], f32, tag="zb")
                            nc.vector.tensor_tensor(zb[:], zp[:], b2_sb[:], add)
                            zin = zb[:]
                        # out := dis * z  (SP ring is idle in layer 2)
                        nc.scalar.activation(o_t[:], zin, AF.Copy,
                                             scale=dis_sb[:, t:t + 1])
                        nc.sync.dma_start(out_d[r0:r1, :], o_t[:])

                # 2-deep software pipeline: PE order is aggp(t), T(t-1),
                # zp(t-2) so the TensorE stream never stalls on the
                # cross-engine transpose round-trip; P built one tile ahead
                p_next = build_p(0)
                aggc_q, aggt_q = {}, {}
                for t in range(ntiles + 2):
                    if t < ntiles:
                        P_cur = p_next
                        p_next = build_p(t + 1) if t + 1 < ntiles else None
                        aggc_q[t] = stage_a(t, P_cur)
                    if 1 <= t <= ntiles:
                        aggt_q[t - 1] = stage_t(t - 1, aggc_q.pop(t - 1))
                    if t >= 2:
                        stage_b(t - 2, aggt_q.pop(t - 2))

    nc.compile()
    return nc


def kernel(x, edge_index, W1, b1, W2, b2):
    x = np.asarray(x, dtype=np.float32)
    W1 = np.asarray(W1, dtype=np.float32)
    W2 = np.asarray(W2, dtype=np.float32)
    b1 = np.asarray(b1, dtype=np.float32)
    b2 = np.asarray(b2, dtype=np.float32)
    meta = _prep_host(x, edge_index)

    has_b1 = bool(np.any(b1))
    has_b2 = bool(np.any(b2))
    nc = _build_program(meta, has_b1, has_b2)

    in_maps = []
    for p in range(NC_CORES):
        m = {
            "g1": meta["g1"][p],
            "xsl": meta["xsl"][p],
            "idx": meta["idx2"][p],
            "toff1": meta["toff1"][p],
            "toff2": meta["toff2"][p],
            "dis": meta["dis_tiles"][p],
            "dis2": meta["dis2_tiles"][p],
            "w1": W1.astype(np.float16), "w2": W2.astype(np.float16),
            "id16": np.eye(128, dtype=np.float16),
            "iota": np.tile(np.arange(128, dtype=np.float16), (128, 1)),
        }
        if has_b1:
            m["b1r"] = np.tile(b1, (128, 1)).astype(np.float32)
        if has_b2:
            m["b2r"] = np.tile(b2, (128, 1)).astype(np.float32)
        in_maps.append(m)

    if os.environ.get("GNN_SIM", "0") == "1":
        from concourse.bass_interp import MultiCoreSim
        sim = MultiCoreSim(nc, num_cores=NC_CORES, trace=False)
        cores = list(sim.cores.values())
        for p, core in enumerate(cores):
            for k, v in in_maps[p].items():
                core.tensor(k)[:] = v
        sim.simulate(check_with_hw=False)
        shards = [cores[p].tensor("out").copy() for p in range(NC_CORES)]
    else:
        from concourse import bass_utils
        trace = os.environ.get("GNN_TRACE", "0") == "1"
        res = bass_utils.run_bass_kernel_spmd(
            nc, in_maps, core_ids=list(range(NC_CORES)), trace=trace)
        if trace and res.exec_time_ns is not None:
            print(f"HW exec time: {res.exec_time_ns} ns")
        kernel.last_results = res
        shards = [res.results[p]["out"] for p in range(NC_CORES)]

    npc = meta["npc"]
    out = np.concatenate([s[:npc] for s in shards], axis=0)
    return out.astype(np.float32)
